# revision 42
# baseline (speedup 1.0000x reference)
"""Multi-head attention on 8 Trainium2 NeuronCores (Bass/Tile), fp8 edition.

Problem: x[2,2048,1024] -> qkv proj (16 heads, hd=64) -> softmax(QK^T/8)V
-> out proj.  mask is all-ones (per spec) and is ignored.

Sharding: core c owns heads {2c, 2c+1} for BOTH batches (tensor-parallel
QKV + attention).  An 8-core AllToAll converts the head-sharded attention
output into a sequence-sharded full-feature activation; core c ends up
with global row chunk c (batch c//4, rows (c%4)*512..) and computes the
output projection full-width with no all-reduce.

Precision/speed strategy (cost model: fp8e4 DoubleRow matmul = 0.5
cycles/row, i.e. 4x bf16 per FLOP at K-pair packing):
  - qk projection: fp8 DoubleRow, 2-term (x8@w8 + xres8@w8) to keep
    q/k noise small; score-side noise averages through softmax.
  - v projection: fp8 DoubleRow, 3-term (x8@wv8 + x8@wvres + xres8@wv8).
  - scores: bf16 (hd=64 contraction can't DoubleRow without expensive
    partition-split copies).
  - exp(scores)->fp8 probabilities on TWO engines: ACT (native Exp, fp8
    out) and DVE (Schraudolph: u8 = round(x*8/ln2 + 55.63) IS the e4m3
    encoding of ~e^x).  GPSIMD can't read PSUM, so it gets no exp work.
  - attn@V: fp8 DoubleRow over key-block pairs, with V stored hi+lo
    (v2hi = fp8(32v), v2lo = fp8(32v - v2hi)) accumulating into the SAME
    PSUM group -> v quantization error cancels.  A ones column in v2hi
    (zeros in v2lo) accumulates the softmax denominator in PSUM row 64.
  - att = 32*num/den in bf16; out projection in plain bf16.
  - v bias and out bias fold into bob = b_out + w_out@bv host-side.

Schedule: normalization finishers (bc matmul + mult + ship chunk) of
each attention group are DEFERRED into the next group's filler slots so
PE never stalls on the recip; output DMAs are split into 128KB chunks
to shrink the serial tail.

PSUM: scores [128,1024] x2 (4 banks) + pso [65,512] x2 (2) + pps
[128,512] x2 (2, also hosts bc broadcasts) = 8 banks.
"""

import numpy as np
import ml_dtypes
from contextlib import ExitStack

import concourse.bass as bass
import concourse.mybir as mybir
import concourse.tile as tile
from concourse import bacc
from concourse.bass_utils import run_bass_kernel_spmd

BF16 = mybir.dt.bfloat16
F32 = mybir.dt.float32
F32R = mybir.dt.float32r
FP8 = mybir.dt.float8e4
U8 = mybir.dt.uint8
NPBF16 = ml_dtypes.bfloat16
NPFP8 = ml_dtypes.float8_e4m3

D, H, HD, B, S = 1024, 16, 64, 2, 2048
NCORES = 8
HPC = 2              # heads per core
FPC = HPC * HD       # 128 features per core
SS = B * S           # 4096 stacked sequence (batch-major)
SC = 512             # output rows per core (post all-to-all)
NKB = S // 128       # 16 key blocks per batch
NPAIR = NKB // 2     # 8 key-block pairs (DoubleRow)
NQC = S // 512       # 4 query chunks per batch
VW = HD + 1          # v columns per head incl. ones column
VP = 80              # padded per-head stride in v2 tiles
WSC = 32.0           # fp8 weight scale

Exp = mybir.ActivationFunctionType.Exp
DR = mybir.MatmulPerfMode.DoubleRow
MUL = mybir.AluOpType.mult
ADD = mybir.AluOpType.add
SUB = mybir.AluOpType.subtract

# Schraudolph exp->e4m3: bits = round(x * 8/ln2 + SCH_B)
SCH_A = 8.0 / np.log(2.0)
SCH_B = 55.63

# per-group exp engine schedule (16 key blocks): a=ACT, d=DVE
EXP_SCHED = "adaadaadaadaadaa"


def _build_nc(with_collective: bool = True):
    nc = bacc.Bacc("TRN2", target_bir_lowering=False, debug=False,
                   num_devices=NCORES)
    xt = nc.dram_tensor("xt", [D, SS], FP8, kind="ExternalInput").ap()
    xres = nc.dram_tensor("xres", [D, SS], FP8, kind="ExternalInput").ap()
    wqk = nc.dram_tensor("wqk", [D, 2 * FPC], FP8, kind="ExternalInput").ap()
    bqk = nc.dram_tensor("bqk", [128, 2], F32, kind="ExternalInput").ap()
    wv = nc.dram_tensor("wv", [D, FPC], FP8, kind="ExternalInput").ap()
    wvr = nc.dram_tensor("wvr", [D, FPC], FP8, kind="ExternalInput").ap()
    wout = nc.dram_tensor("wout", [D, D], BF16, kind="ExternalInput").ap()
    bob = nc.dram_tensor("bob", [128, D], F32, kind="ExternalInput").ap()
    ident = nc.dram_tensor("ident", [128, 128], BF16,
                           kind="ExternalInput").ap()
    onesr = nc.dram_tensor("onesr", [VW, HD], F32R,
                           kind="ExternalInput").ap()
    out = nc.dram_tensor("out", [SC, D], F32, kind="ExternalOutput").ap()

    with ExitStack() as ctx:
        tc = ctx.enter_context(tile.TileContext(nc))
        persist = ctx.enter_context(tc.tile_pool(name="persist", bufs=1))
        pexp = ctx.enter_context(tc.tile_pool(name="pexp", bufs=4))
        pwork = ctx.enter_context(tc.tile_pool(name="pwork", bufs=4))
        pbig = ctx.enter_context(tc.tile_pool(name="pbig", bufs=2, space="PSUM"))
        pso = ctx.enter_context(tc.tile_pool(name="pso", bufs=2, space="PSUM"))
        pps = ctx.enter_context(tc.tile_pool(name="pps", bufs=2, space="PSUM"))
        dram = ctx.enter_context(tc.tile_pool(name="dram", bufs=2, space="DRAM"))

        # ---------------- persistent SBUF ----------------
        # xt_sb[b]: [128, (i 8, s 2048)] fp8; xr_sb = fp8 residual
        xt_sb = [persist.tile([128, 8 * S], FP8, tag=f"xt{b}",
                              name=f"xt{b}") for b in range(B)]
        xt_r = [t.rearrange("p (i s) -> p i s", i=8) for t in xt_sb]
        xr_sb = [persist.tile([128, 8 * S], FP8, tag=f"xr{b}",
                              name=f"xr{b}") for b in range(B)]
        xr_r = [t.rearrange("p (i s) -> p i s", i=8) for t in xr_sb]
        wqk_sb = persist.tile([128, 8 * 2 * FPC], FP8, tag="wqk",
                              name="wqk_sb")
        wqk_r = wqk_sb.rearrange("p (i f) -> p i f", i=8)
        bqk_sb = persist.tile([128, 2], F32, tag="bqk", name="bqk")
        wv_sb = persist.tile([128, 8 * FPC], FP8, tag="wv", name="wv_sb")
        wv_r = wv_sb.rearrange("p (i f) -> p i f", i=8)
        wvr_sb = persist.tile([128, 8 * FPC], FP8, tag="wvr", name="wvr_sb")
        wvr_r = wvr_sb.rearrange("p (i f) -> p i f", i=8)
        wout_sb = persist.tile([128, 8 * D], BF16, tag="wout", name="wout_sb")
        wout_r = wout_sb.rearrange("p (i f) -> p i f", i=8)
        bob_sb = persist.tile([128, D], F32, tag="bob", name="bob")
        ident_sb = persist.tile([128, 128], BF16, tag="ident", name="ident")
        qk_sb = [[[persist.tile([128, 512], BF16, tag=f"qk{b}_{m}_{qn}",
                                name=f"qk{b}_{m}_{qn}") for qn in range(NQC)]
                  for m in range(2)]
                 for b in range(B)]
        # v2{hi,lo}[b][pi]: [128, 2*(2*VP)] fp8 - key-block pair pi; layout
        # [p, j(2), h(2), c(VP)], v at c 0:64; ones col 64: 1.0 in hi, 0 in lo
        v2hi = [[persist.tile([128, 2 * 2 * VP], FP8, tag=f"vh{b}_{pi}",
                              name=f"vh{b}_{pi}") for pi in range(NPAIR)]
                for b in range(B)]
        v2lo = [[persist.tile([128, 2 * 2 * VP], FP8, tag=f"vl{b}_{pi}",
                              name=f"vl{b}_{pi}") for pi in range(NPAIR)]
                for b in range(B)]
        v2hi_r = [[t.rearrange("p (j h c) -> p j h c", j=2, h=2)
                   for t in row] for row in v2hi]
        v2lo_r = [[t.rearrange("p (j h c) -> p j h c", j=2, h=2)
                   for t in row] for row in v2lo]
        att_sb = [[persist.tile([64, S], BF16, tag=f"att{b}_{h}",
                                name=f"att{b}_{h}") for h in range(HPC)]
                  for b in range(B)]
        ones32 = persist.tile([VW, HD], F32R, tag="ones32", name="ones32")
        nc.sync.dma_start(out=ones32, in_=onesr[:, :])
        ao_sb = [persist.tile([128, SC], BF16, tag=f"ao{i}", name=f"ao{i}")
                 for i in range(8)]
        part_sb = [persist.tile([128, 512], F32, tag=f"part{t}",
                                name=f"part{t}") for t in range(8)]
        for b in range(B):
            for pi in range(NPAIR):
                nc.gpsimd.memset(v2hi_r[b][pi][:, :, :, HD:VW], 1.0)
                nc.gpsimd.memset(v2lo_r[b][pi][:, :, :, HD:VW], 0.0)

        # ---------------- loads (ordered by first use) ----------------
        def load_x(dst_r, src, b, s0, s1, eng=None):
            # (p, i, s) -> src[i*128 + p, b*S + s]
            ap = bass.AP(tensor=src.tensor, offset=b * S + s0,
                         ap=[[SS, 128], [128 * SS, 8], [1, s1 - s0]])
            (eng or nc.sync).dma_start(out=dst_r[b][:, :, s0:s1], in_=ap)

        wqk_src = bass.AP(tensor=wqk.tensor, offset=0,
                          ap=[[2 * FPC, 128], [128 * 2 * FPC, 8], [1, 2 * FPC]])
        nc.sync.dma_start(out=wqk_r, in_=wqk_src)
        load_x(xt_r, xt, 0, 0, 512)
        load_x(xr_r, xres, 0, 0, 512, eng=nc.gpsimd)
        nc.sync.dma_start(out=bqk_sb, in_=bqk[:, :])
        load_x(xt_r, xt, 0, 512, 2048)
        load_x(xr_r, xres, 0, 512, 2048, eng=nc.gpsimd)
        wv_src = bass.AP(tensor=wv.tensor, offset=0,
                         ap=[[FPC, 128], [128 * FPC, 8], [1, FPC]])
        nc.sync.dma_start(out=wv_r, in_=wv_src)
        wvr_src = bass.AP(tensor=wvr.tensor, offset=0,
                          ap=[[FPC, 128], [128 * FPC, 8], [1, FPC]])
        nc.sync.dma_start(out=wvr_r, in_=wvr_src)
        nc.sync.dma_start(out=ident_sb, in_=ident[:, :])
        load_x(xt_r, xt, 1, 0, 2048)
        load_x(xr_r, xres, 1, 0, 2048, eng=nc.gpsimd)
        wout_src = bass.AP(tensor=wout.tensor, offset=0,
                           ap=[[D, 128], [128 * D, 8], [1, D]])
        nc.sync.dma_start(out=wout_r, in_=wout_src)
        nc.sync.dma_start(out=bob_sb, in_=bob[:, :])

        a2a_in = [dram.tile([8, HD, SC], BF16, tag=f"a2a_in{h}",
                            name=f"a2a_in{h}", bufs=1) for h in range(HPC)]
        a2a_out = [dram.tile([8, HD, SC], BF16, tag=f"a2a_out{h}",
                             name=f"a2a_out{h}", bufs=1) for h in range(HPC)]

        def emit_a2a(h):
            if with_collective:
                nc.gpsimd.collective_compute(
                    "AllToAll", mybir.AluOpType.bypass,
                    replica_groups=[list(range(8))],
                    ins=[a2a_in[h][:, :, :].opt()],
                    outs=[a2a_out[h][:, :, :].opt()])

        # ------------- projections + attention, interleaved -------------
        def emit_qk(b, m, qn):
            ps = pps.tile([128, 512], F32, tag="ps", name="ps_qk")
            for i in range(4):
                nc.tensor.matmul(
                    ps,
                    wqk_r[:, 2 * i:2 * i + 2, m * 128:(m + 1) * 128],
                    xt_r[b][:, 2 * i:2 * i + 2, qn * 512:(qn + 1) * 512],
                    start=(i == 0), stop=False, perf_mode=DR)
            for i in range(4):
                nc.tensor.matmul(
                    ps,
                    wqk_r[:, 2 * i:2 * i + 2, m * 128:(m + 1) * 128],
                    xr_r[b][:, 2 * i:2 * i + 2, qn * 512:(qn + 1) * 512],
                    start=False, stop=(i == 3), perf_mode=DR)
            nc.vector.tensor_scalar(
                qk_sb[b][m][qn], ps,
                (0.125 / WSC) if m == 0 else (1.0 / WSC),
                bqk_sb[:, m:m + 1], op0=MUL, op1=ADD)

        # v projection, operand-swapped: out [128 feat, 512 seq] in one
        # 12-matmul pass per (b, cq); transposed back per 128-seq block
        vfm_store = {}

        def emit_v_mm(b, cq):
            ps = pps.tile([128, 512], F32, tag="ps", name="ps_v")
            terms = [(wv_r, xt_r), (wvr_r, xt_r), (wv_r, xr_r)]
            n = 0
            for wsrc, xsrc in terms:
                for i in range(4):
                    nc.tensor.matmul(
                        ps,
                        wsrc[:, 2 * i:2 * i + 2, :],
                        xsrc[b][:, 2 * i:2 * i + 2,
                                cq * 512:(cq + 1) * 512],
                        start=(n == 0), stop=(n == 11), perf_mode=DR)
                    n += 1
            vfm = pwork.tile([128, 512], BF16, tag="vfm", name="vfm")
            if (b * 4 + cq) % 2:
                nc.scalar.copy(vfm, ps)
            else:
                nc.vector.tensor_copy(vfm, ps)
            vfm_store[(b, cq)] = vfm

        def emit_v_tr(b, cq):
            vfm = vfm_store.pop((b, cq))
            for s4 in range(4):
                sn = cq * 4 + s4
                pi, j = sn // 2, sn % 2
                pst = pps.tile([128, 128], BF16, tag="ps", name="pst")
                nc.tensor.transpose(
                    pst, vfm[:, s4 * 128:(s4 + 1) * 128], ident_sb)
                pst_r = pst.rearrange("p (h c) -> p h c", h=2)
                hi = v2hi_r[b][pi][:, j, :, 0:HD]
                nc.vector.tensor_copy(hi, pst_r)
                nc.vector.tensor_tensor(
                    v2lo_r[b][pi][:, j, :, 0:HD], pst_r, hi, op=SUB)

        def emit_attn(b, h, qh, fillers=()):
            # fillers: [(slot, thunk)]; popped at kb >= slot, so a filler's
            # products may only be consumed at kb >= slot (or later groups)
            fillers = sorted([e if isinstance(e, tuple) else (0, e)
                              for e in fillers], key=lambda e: e[0])
            pb = h * 64
            ps_o = [pso.tile([VW, 512], F32, tag="pso",
                             name=f"ps_o{q2}") for q2 in range(2)]

            def emit_av(pi, exv):
                for q2 in range(2):
                    nc.tensor.matmul(
                        ps_o[q2],
                        v2hi_r[b][pi][:, :, h, 0:VW],
                        exv[:, :, q2 * 512:(q2 + 1) * 512],
                        start=(pi == 0), stop=False, perf_mode=DR)
                    nc.tensor.matmul(
                        ps_o[q2],
                        v2lo_r[b][pi][:, :, h, 0:VW],
                        exv[:, :, q2 * 512:(q2 + 1) * 512],
                        start=False, stop=(pi == NPAIR - 1), perf_mode=DR)

            # attn@V for pair pi is emitted one pair late so PE never
            # hard-waits on the pair's last exp
            pend = None
            for pi in range(NPAIR):
                ex = pexp.tile([128, 2 * 1024], FP8, tag="expT",
                               name="expT")
                exv = ex.rearrange("p (j q) -> p j q", j=2)
                exu = ex.bitcast(U8).rearrange("p (j q) -> p j q", j=2)
                for j in range(2):
                    kb = 2 * pi + j
                    while fillers and fillers[0][0] <= kb:
                        fillers.pop(0)[1]()
                    ps_s = pbig.tile([128, 1024], F32, tag="scores",
                                     name="ps_s")
                    for q2 in range(2):
                        qc = qh * 2 + q2
                        nc.tensor.matmul(
                            ps_s[:, q2 * 512:(q2 + 1) * 512],
                            qk_sb[b][1][kb // 4][pb:pb + 64,
                                                 (kb % 4) * 128:
                                                 (kb % 4 + 1) * 128],
                            qk_sb[b][0][qc][pb:pb + 64, :],
                            start=True, stop=True)
                    if EXP_SCHED[kb] == "a":
                        nc.scalar.activation(exv[:, j, :], ps_s, Exp)
                    else:
                        nc.vector.tensor_scalar(
                            exu[:, j, :], ps_s, SCH_A, SCH_B,
                            op0=MUL, op1=ADD)
                    if j == 1 and pend is not None:
                        emit_av(*pend)
                pend = (pi, exv)
            emit_av(*pend)
            for _, f in fillers:
                f()
            # normalization head: recip + otmp now; bc+mult+ship deferred
            rec_s = pwork.tile([VW, 1024], F32R, tag="rec", name="rec_s")
            with nc.allow_low_precision(
                    reason="softmax denom recip rounded to f32r "
                           "for the PE broadcast"):
                for q2 in range(2):
                    nc.vector.reciprocal(
                        rec_s[HD:VW, q2 * 512:(q2 + 1) * 512],
                        ps_o[q2][HD:VW, :])
            otmps = []
            for q2 in range(2):
                otmp = pwork.tile([HD, 512], F32, tag="otmp",
                                  name="otmp")
                nc.scalar.copy(otmp, ps_o[q2][0:HD, :])
                otmps.append(otmp)

            def finish(q2):
                qc = qh * 2 + q2
                bc_ps = pps.tile([HD, 512], F32, tag="ps", name="bc_ps")
                nc.tensor.matmul(
                    bc_ps,
                    ones32[HD:VW, :],
                    rec_s[HD:VW, q2 * 512:(q2 + 1) * 512],
                    start=True, stop=True)
                nc.vector.tensor_tensor(
                    att_sb[b][h][:, qc * 512:(qc + 1) * 512],
                    otmps[q2], bc_ps[:, :], op=MUL)
                if qh == 1 and q2 == 1:
                    # (b,h) complete: ship the whole row in one DMA
                    nc.sync.dma_start(
                        out=a2a_in[h][b * 4:(b + 1) * 4, :, :].rearrange(
                            "j p s -> p j s"),
                        in_=att_sb[b][h].rearrange("p (j s) -> p j s", j=4))

            return [lambda: finish(0), lambda: finish(1)]

        srcb = a2a_out if with_collective else a2a_in

        def emit_unload(hi):
            for jj in range(4):
                eng = nc.sync if jj % 2 == 0 else nc.gpsimd
                eng.dma_start(
                    out=ao_sb[4 * hi + jj],
                    in_=srcb[hi][2 * jj:2 * jj + 2, :, :].rearrange(
                        "j p s -> (j p) s"))

        # output projection tile t = sm*2 + en (bf16)
        def emit_out1(t):
            sm, en = t // 2, t % 2
            ps = pps.tile([128, 512], F32, tag="ps", name="ps_out1")
            for kk in range(4):
                nc.tensor.matmul(
                    ps, ao_sb[kk][:, sm * 128:(sm + 1) * 128],
                    wout_sb[:, kk * D + en * 512:kk * D + (en + 1) * 512],
                    start=(kk == 0), stop=(kk == 3))
            nc.vector.scalar_tensor_tensor(
                part_sb[t], ps, 1.0 / WSC,
                bob_sb[:, en * 512:(en + 1) * 512], op0=MUL, op1=ADD)

        def emit_out2(t):
            sm, en = t // 2, t % 2
            ps = pps.tile([128, 512], F32, tag="ps", name="ps_out2")
            for kk in range(4, 8):
                nc.tensor.matmul(
                    ps, ao_sb[kk][:, sm * 128:(sm + 1) * 128],
                    wout_sb[:, kk * D + en * 512:kk * D + (en + 1) * 512],
                    start=(kk == 4), stop=(kk == 7))
            osb = pwork.tile([128, 512], F32, tag="outsb", name="osb")
            nc.vector.scalar_tensor_tensor(
                osb, ps, 1.0 / WSC, part_sb[t], op0=MUL, op1=ADD)
            eng = nc.sync if t % 2 == 0 else nc.gpsimd
            eng.dma_start(
                out=out[sm * 128:(sm + 1) * 128, en * 512:(en + 1) * 512],
                in_=osb)

        def F(fn, *a):
            return lambda: fn(*a)

        # Filler safety rule: a filler popped at kb-slot i is emitted just
        # before slot i's scores, so anything it produces may only be
        # consumed at kb >= i (or by a later group).
        emit_qk(0, 1, 0)
        emit_qk(0, 1, 1)
        emit_v_mm(0, 0)
        emit_qk(0, 0, 0)
        emit_v_tr(0, 0)
        emit_qk(0, 0, 1)
        emit_v_mm(0, 1)
        fin = emit_attn(0, 0, 0, fillers=(
            [(0, F(emit_v_tr, 0, 1)),
             (2, F(emit_v_mm, 0, 2)), (4, F(emit_v_tr, 0, 2)),
             (6, F(emit_qk, 0, 1, 2)),
             (8, F(emit_v_mm, 0, 3)), (10, F(emit_v_tr, 0, 3)),
             (11, F(emit_qk, 0, 1, 3)),
             (13, F(emit_qk, 0, 0, 2)), (15, F(emit_qk, 0, 0, 3))]))
        fin = emit_attn(0, 1, 0, fillers=[(1, fin[0]), (3, fin[1])])
        fin = emit_attn(0, 0, 1, fillers=(
            [(1, fin[0]), (3, fin[1])]
            + [(i * 2, F(emit_qk, 1, 1, qn))
               for i, qn in enumerate(range(NQC))]
            + [(8, F(emit_qk, 1, 0, 0)), (10, F(emit_qk, 1, 0, 1))]))
        fin = emit_attn(0, 1, 1, fillers=(
            [(1, fin[0]), (3, fin[1])]
            + [(0, F(emit_v_mm, 1, 0)), (2, F(emit_v_tr, 1, 0)),
               (4, F(emit_v_mm, 1, 1)), (6, F(emit_v_tr, 1, 1))]
            + [(14, F(emit_qk, 1, 0, 2)), (15, F(emit_qk, 1, 0, 3))]))
        fin = emit_attn(1, 0, 0, fillers=(
            [(1, fin[0]), (3, fin[1])]
            + [(0, F(emit_v_mm, 1, 2)), (2, F(emit_v_tr, 1, 2)),
               (4, F(emit_v_mm, 1, 3)), (6, F(emit_v_tr, 1, 3))]))
        fin = emit_attn(1, 0, 1, fillers=[(1, fin[0]), (3, fin[1])])
        # (1,1,0) flushes the last even-head finishers -> a2a #0 can go
        fin = emit_attn(1, 1, 0, fillers=[(1, fin[0]), (3, fin[1])])
        emit_a2a(0)
        emit_unload(0)
        fin2 = emit_attn(1, 1, 1, fillers=(
            [(1, fin[0]), (3, fin[1])]
            + [(2 * t, F(emit_out1, t)) for t in range(8)]))
        for f in fin2:
            f()
        emit_a2a(1)
        emit_unload(1)
        for t in range(8):
            emit_out2(t)

    nc.compile()
    return nc


_NC_CACHE = {}


def _get_nc(with_collective: bool = True):
    key = bool(with_collective)
    if key not in _NC_CACHE:
        _NC_CACHE[key] = _build_nc(with_collective)
    return _NC_CACHE[key]


def make_in_maps(x, w_qkv, b_qkv, w_out, b_out):
    """Host-side sharding/prep. Returns per-core input dicts."""
    x = np.asarray(x, dtype=np.float32)
    w_qkv = np.asarray(w_qkv, dtype=np.float32)
    b_qkv = np.asarray(b_qkv, dtype=np.float32)
    w_out = np.asarray(w_out, dtype=np.float32)
    b_out = np.asarray(b_out, dtype=np.float32)

    wq = w_qkv[0:D].reshape(H, HD, D)
    wk = w_qkv[D:2 * D].reshape(H, HD, D)
    wv = w_qkv[2 * D:3 * D].reshape(H, HD, D)
    bq = b_qkv[0:D].reshape(H, HD)
    bk = b_qkv[D:2 * D].reshape(H, HD)
    scale = 1.0 / np.sqrt(HD)

    perm = np.concatenate(
        [np.arange(h * HD, (h + 1) * HD) for h in range(0, H, 2)]
        + [np.arange(h * HD, (h + 1) * HD) for h in range(1, H, 2)])
    wout_t = np.ascontiguousarray(w_out.T[perm]).astype(NPBF16)
    # v bias passes through attention unchanged (softmax rows sum to 1),
    # so its out-proj image folds into the output bias
    bv_full = b_qkv[2 * D:3 * D]
    bob_vec = b_out + w_out @ bv_full
    bob = np.ascontiguousarray(
        np.broadcast_to(bob_vec, (128, D))).astype(np.float32)

    # [d, 4096] stacked batch-major; fp8 hi + fp8 residual
    xt_f32 = np.ascontiguousarray(
        np.concatenate([x[0].T, x[1].T], axis=1))
    xt_all = xt_f32.astype(NPFP8)
    xres_all = (xt_f32 - xt_all.astype(np.float32)).astype(NPFP8)

    in_maps = []
    for c in range(NCORES):
        hs = slice(c * HPC, (c + 1) * HPC)
        wq_c = (wq[hs].reshape(FPC, D) * WSC).T
        wk_c = (wk[hs].reshape(FPC, D) * WSC).T
        wqk_c = np.concatenate([wq_c, wk_c], axis=1).astype(NPFP8)
        bqk_c = np.concatenate([bq[hs].reshape(FPC) * scale,
                                bk[hs].reshape(FPC)])
        bqk_c = np.ascontiguousarray(
            bqk_c.reshape(2, 128).T).astype(np.float32)
        wv_f = (wv[hs].reshape(FPC, D) * WSC).T
        wv_c = wv_f.astype(NPFP8)
        wvr_c = (wv_f - wv_c.astype(np.float32)).astype(NPFP8)
        in_maps.append({
            "onesr": np.ones((VW, HD), dtype=np.float32),
            "ident": np.eye(128, dtype=np.float32).astype(NPBF16),
            "xt": xt_all,
            "xres": xres_all,
            "wqk": np.ascontiguousarray(wqk_c),
            "bqk": bqk_c,
            "wv": np.ascontiguousarray(wv_c),
            "wvr": np.ascontiguousarray(wvr_c),
            "wout": wout_t,
            "bob": bob,
        })
    return in_maps


def assemble_output(results):
    out = np.empty((B, S, D), dtype=np.float32)
    for c in range(NCORES):
        b, sg = c // 4, c % 4
        out[b, sg * SC:(sg + 1) * SC, :] = results[c]["out"]
    return out


def kernel(x, mask, w_qkv, b_qkv, w_out, b_out):
    nc = _get_nc(True)
    in_maps = make_in_maps(x, w_qkv, b_qkv, w_out, b_out)
    res = run_bass_kernel_spmd(nc, in_maps, core_ids=list(range(NCORES)))
    return assemble_output(res.results)


# revision 43
# speedup vs baseline: 1.0076x; 1.0076x over previous
"""Multi-head attention on 8 Trainium2 NeuronCores (Bass/Tile), fp8 edition.

Problem: x[2,2048,1024] -> qkv proj (16 heads, hd=64) -> softmax(QK^T/8)V
-> out proj.  mask is all-ones (per spec) and is ignored.

Sharding: core c owns heads {2c, 2c+1} for BOTH batches (tensor-parallel
QKV + attention).  An 8-core AllToAll converts the head-sharded attention
output into a sequence-sharded full-feature activation; core c ends up
with global row chunk c (batch c//4, rows (c%4)*512..) and computes the
output projection full-width with no all-reduce.

Precision/speed strategy (cost model: fp8e4 DoubleRow matmul = 0.5
cycles/row, i.e. 4x bf16 per FLOP at K-pair packing):
  - qk projection: fp8 DoubleRow, 2-term (x8@w8 + xres8@w8) to keep
    q/k noise small; score-side noise averages through softmax.
  - v projection: fp8 DoubleRow, 3-term (x8@wv8 + x8@wvres + xres8@wv8).
  - scores: bf16 (hd=64 contraction can't DoubleRow without expensive
    partition-split copies).
  - exp(scores)->fp8 probabilities on TWO engines: ACT (native Exp, fp8
    out) and DVE (Schraudolph: u8 = round(x*8/ln2 + 55.63) IS the e4m3
    encoding of ~e^x).  GPSIMD can't read PSUM, so it gets no exp work.
  - attn@V: fp8 DoubleRow over key-block pairs, with V stored hi+lo
    (v2hi = fp8(32v), v2lo = fp8(32v - v2hi)) accumulating into the SAME
    PSUM group -> v quantization error cancels.  A ones column in v2hi
    (zeros in v2lo) accumulates the softmax denominator in PSUM row 64.
  - att = 32*num/den in bf16; out projection in plain bf16.
  - v bias and out bias fold into bob = b_out + w_out@bv host-side.

Schedule: normalization finishers (bc matmul + mult + ship chunk) of
each attention group are DEFERRED into the next group's filler slots so
PE never stalls on the recip; output DMAs are split into 128KB chunks
to shrink the serial tail.

PSUM: scores [128,1024] x2 (4 banks) + pso [65,512] x2 (2) + pps
[128,512] x2 (2, also hosts bc broadcasts) = 8 banks.
"""

import numpy as np
import ml_dtypes
from contextlib import ExitStack

import concourse.bass as bass
import concourse.mybir as mybir
import concourse.tile as tile
from concourse import bacc
from concourse.bass_utils import run_bass_kernel_spmd

BF16 = mybir.dt.bfloat16
F32 = mybir.dt.float32
F32R = mybir.dt.float32r
FP8 = mybir.dt.float8e4
U8 = mybir.dt.uint8
NPBF16 = ml_dtypes.bfloat16
NPFP8 = ml_dtypes.float8_e4m3

D, H, HD, B, S = 1024, 16, 64, 2, 2048
NCORES = 8
HPC = 2              # heads per core
FPC = HPC * HD       # 128 features per core
SS = B * S           # 4096 stacked sequence (batch-major)
SC = 512             # output rows per core (post all-to-all)
NKB = S // 128       # 16 key blocks per batch
NPAIR = NKB // 2     # 8 key-block pairs (DoubleRow)
NQC = S // 512       # 4 query chunks per batch
VW = HD + 1          # v columns per head incl. ones column
VP = 80              # padded per-head stride in v2 tiles
WSC = 32.0           # fp8 weight scale

Exp = mybir.ActivationFunctionType.Exp
DR = mybir.MatmulPerfMode.DoubleRow
MUL = mybir.AluOpType.mult
ADD = mybir.AluOpType.add
SUB = mybir.AluOpType.subtract

# Schraudolph exp->e4m3: bits = round(x * 8/ln2 + SCH_B)
SCH_A = 8.0 / np.log(2.0)
SCH_B = 55.63

# per-group exp engine schedule (16 key blocks): a=ACT, d=DVE
EXP_SCHED = "adaadaadaadaadaa"


def _build_nc(with_collective: bool = True):
    nc = bacc.Bacc("TRN2", target_bir_lowering=False, debug=False,
                   num_devices=NCORES)
    xt = nc.dram_tensor("xt", [D, SS], FP8, kind="ExternalInput").ap()
    xres = nc.dram_tensor("xres", [D, SS], FP8, kind="ExternalInput").ap()
    wqk = nc.dram_tensor("wqk", [D, 2 * FPC], FP8, kind="ExternalInput").ap()
    bqk = nc.dram_tensor("bqk", [128, 2], F32, kind="ExternalInput").ap()
    wv = nc.dram_tensor("wv", [D, FPC], FP8, kind="ExternalInput").ap()
    wvr = nc.dram_tensor("wvr", [D, FPC], FP8, kind="ExternalInput").ap()
    wout = nc.dram_tensor("wout", [D, D], BF16, kind="ExternalInput").ap()
    bob = nc.dram_tensor("bob", [128, D], F32, kind="ExternalInput").ap()
    ident = nc.dram_tensor("ident", [128, 128], BF16,
                           kind="ExternalInput").ap()
    onesr = nc.dram_tensor("onesr", [VW, HD], F32R,
                           kind="ExternalInput").ap()
    out = nc.dram_tensor("out", [SC, D], F32, kind="ExternalOutput").ap()

    with ExitStack() as ctx:
        tc = ctx.enter_context(tile.TileContext(nc))
        persist = ctx.enter_context(tc.tile_pool(name="persist", bufs=1))
        pexp = ctx.enter_context(tc.tile_pool(name="pexp", bufs=4))
        pwork = ctx.enter_context(tc.tile_pool(name="pwork", bufs=4))
        pbig = ctx.enter_context(tc.tile_pool(name="pbig", bufs=2, space="PSUM"))
        pso = ctx.enter_context(tc.tile_pool(name="pso", bufs=2, space="PSUM"))
        pps = ctx.enter_context(tc.tile_pool(name="pps", bufs=2, space="PSUM"))
        dram = ctx.enter_context(tc.tile_pool(name="dram", bufs=2, space="DRAM"))

        # ---------------- persistent SBUF ----------------
        # xt_sb[b]: [128, (i 8, s 2048)] fp8; xr_sb = fp8 residual
        xt_sb = [persist.tile([128, 8 * S], FP8, tag=f"xt{b}",
                              name=f"xt{b}") for b in range(B)]
        xt_r = [t.rearrange("p (i s) -> p i s", i=8) for t in xt_sb]
        xr_sb = [persist.tile([128, 8 * S], FP8, tag=f"xr{b}",
                              name=f"xr{b}") for b in range(B)]
        xr_r = [t.rearrange("p (i s) -> p i s", i=8) for t in xr_sb]
        wqk_sb = persist.tile([128, 8 * 2 * FPC], FP8, tag="wqk",
                              name="wqk_sb")
        wqk_r = wqk_sb.rearrange("p (i f) -> p i f", i=8)
        bqk_sb = persist.tile([128, 2], F32, tag="bqk", name="bqk")
        wv_sb = persist.tile([128, 8 * FPC], FP8, tag="wv", name="wv_sb")
        wv_r = wv_sb.rearrange("p (i f) -> p i f", i=8)
        wvr_sb = persist.tile([128, 8 * FPC], FP8, tag="wvr", name="wvr_sb")
        wvr_r = wvr_sb.rearrange("p (i f) -> p i f", i=8)
        wout_sb = persist.tile([128, 8 * D], BF16, tag="wout", name="wout_sb")
        wout_r = wout_sb.rearrange("p (i f) -> p i f", i=8)
        bob_sb = persist.tile([128, D], F32, tag="bob", name="bob")
        ident_sb = persist.tile([128, 128], BF16, tag="ident", name="ident")
        qk_sb = [[[persist.tile([128, 512], BF16, tag=f"qk{b}_{m}_{qn}",
                                name=f"qk{b}_{m}_{qn}") for qn in range(NQC)]
                  for m in range(2)]
                 for b in range(B)]
        # v2{hi,lo}[b][pi]: [128, 2*(2*VP)] fp8 - key-block pair pi; layout
        # [p, j(2), h(2), c(VP)], v at c 0:64; ones col 64: 1.0 in hi, 0 in lo
        v2hi = [[persist.tile([128, 2 * 2 * VP], FP8, tag=f"vh{b}_{pi}",
                              name=f"vh{b}_{pi}") for pi in range(NPAIR)]
                for b in range(B)]
        v2lo = [[persist.tile([128, 2 * 2 * VP], FP8, tag=f"vl{b}_{pi}",
                              name=f"vl{b}_{pi}") for pi in range(NPAIR)]
                for b in range(B)]
        v2hi_r = [[t.rearrange("p (j h c) -> p j h c", j=2, h=2)
                   for t in row] for row in v2hi]
        v2lo_r = [[t.rearrange("p (j h c) -> p j h c", j=2, h=2)
                   for t in row] for row in v2lo]
        att_sb = [[persist.tile([64, S], BF16, tag=f"att{b}_{h}",
                                name=f"att{b}_{h}") for h in range(HPC)]
                  for b in range(B)]
        ones32 = persist.tile([VW, HD], F32R, tag="ones32", name="ones32")
        nc.sync.dma_start(out=ones32, in_=onesr[:, :])
        ao_sb = [persist.tile([128, SC], BF16, tag=f"ao{i}", name=f"ao{i}")
                 for i in range(8)]
        part_sb = [persist.tile([128, 512], F32, tag=f"part{t}",
                                name=f"part{t}") for t in range(8)]
        for b in range(B):
            for pi in range(NPAIR):
                nc.gpsimd.memset(v2hi_r[b][pi][:, :, :, HD:VW], 1.0)
                nc.gpsimd.memset(v2lo_r[b][pi][:, :, :, HD:VW], 0.0)

        # ---------------- loads (ordered by first use) ----------------
        def load_x(dst_r, src, b, s0, s1, eng=None):
            # (p, i, s) -> src[i*128 + p, b*S + s]
            ap = bass.AP(tensor=src.tensor, offset=b * S + s0,
                         ap=[[SS, 128], [128 * SS, 8], [1, s1 - s0]])
            (eng or nc.sync).dma_start(out=dst_r[b][:, :, s0:s1], in_=ap)

        wqk_src = bass.AP(tensor=wqk.tensor, offset=0,
                          ap=[[2 * FPC, 128], [128 * 2 * FPC, 8], [1, 2 * FPC]])
        nc.sync.dma_start(out=wqk_r, in_=wqk_src)
        load_x(xt_r, xt, 0, 0, 512)
        load_x(xr_r, xres, 0, 0, 512)
        nc.sync.dma_start(out=bqk_sb, in_=bqk[:, :])
        load_x(xt_r, xt, 0, 512, 2048)
        load_x(xr_r, xres, 0, 512, 2048)
        wv_src = bass.AP(tensor=wv.tensor, offset=0,
                         ap=[[FPC, 128], [128 * FPC, 8], [1, FPC]])
        nc.sync.dma_start(out=wv_r, in_=wv_src)
        wvr_src = bass.AP(tensor=wvr.tensor, offset=0,
                          ap=[[FPC, 128], [128 * FPC, 8], [1, FPC]])
        nc.sync.dma_start(out=wvr_r, in_=wvr_src)
        nc.sync.dma_start(out=ident_sb, in_=ident[:, :])
        load_x(xt_r, xt, 1, 0, 2048)
        load_x(xr_r, xres, 1, 0, 2048)
        wout_src = bass.AP(tensor=wout.tensor, offset=0,
                           ap=[[D, 128], [128 * D, 8], [1, D]])
        nc.sync.dma_start(out=wout_r, in_=wout_src)
        nc.sync.dma_start(out=bob_sb, in_=bob[:, :])

        a2a_in = [dram.tile([8, HD, SC], BF16, tag=f"a2a_in{h}",
                            name=f"a2a_in{h}", bufs=1) for h in range(HPC)]
        a2a_out = [dram.tile([8, HD, SC], BF16, tag=f"a2a_out{h}",
                             name=f"a2a_out{h}", bufs=1) for h in range(HPC)]

        def emit_a2a(h):
            if with_collective:
                nc.gpsimd.collective_compute(
                    "AllToAll", mybir.AluOpType.bypass,
                    replica_groups=[list(range(8))],
                    ins=[a2a_in[h][:, :, :].opt()],
                    outs=[a2a_out[h][:, :, :].opt()])

        # ------------- projections + attention, interleaved -------------
        def emit_qk(b, m, qn):
            ps = pps.tile([128, 512], F32, tag="ps", name="ps_qk")
            for i in range(4):
                nc.tensor.matmul(
                    ps,
                    wqk_r[:, 2 * i:2 * i + 2, m * 128:(m + 1) * 128],
                    xt_r[b][:, 2 * i:2 * i + 2, qn * 512:(qn + 1) * 512],
                    start=(i == 0), stop=False, perf_mode=DR)
            for i in range(4):
                nc.tensor.matmul(
                    ps,
                    wqk_r[:, 2 * i:2 * i + 2, m * 128:(m + 1) * 128],
                    xr_r[b][:, 2 * i:2 * i + 2, qn * 512:(qn + 1) * 512],
                    start=False, stop=(i == 3), perf_mode=DR)
            nc.vector.tensor_scalar(
                qk_sb[b][m][qn], ps,
                (0.125 / WSC) if m == 0 else (1.0 / WSC),
                bqk_sb[:, m:m + 1], op0=MUL, op1=ADD)

        # v projection, operand-swapped: out [128 feat, 512 seq] in one
        # 12-matmul pass per (b, cq); transposed back per 128-seq block
        vfm_store = {}

        def emit_v_mm(b, cq):
            ps = pps.tile([128, 512], F32, tag="ps", name="ps_v")
            terms = [(wv_r, xt_r), (wvr_r, xt_r), (wv_r, xr_r)]
            n = 0
            for wsrc, xsrc in terms:
                for i in range(4):
                    nc.tensor.matmul(
                        ps,
                        wsrc[:, 2 * i:2 * i + 2, :],
                        xsrc[b][:, 2 * i:2 * i + 2,
                                cq * 512:(cq + 1) * 512],
                        start=(n == 0), stop=(n == 11), perf_mode=DR)
                    n += 1
            vfm = pwork.tile([128, 512], BF16, tag="vfm", name="vfm")
            if (b * 4 + cq) % 2:
                nc.scalar.copy(vfm, ps)
            else:
                nc.vector.tensor_copy(vfm, ps)
            vfm_store[(b, cq)] = vfm

        def emit_v_tr(b, cq):
            vfm = vfm_store.pop((b, cq))
            for s4 in range(4):
                sn = cq * 4 + s4
                pi, j = sn // 2, sn % 2
                pst = pps.tile([128, 128], BF16, tag="ps", name="pst")
                nc.tensor.transpose(
                    pst, vfm[:, s4 * 128:(s4 + 1) * 128], ident_sb)
                pst_r = pst.rearrange("p (h c) -> p h c", h=2)
                hi = v2hi_r[b][pi][:, j, :, 0:HD]
                nc.vector.tensor_copy(hi, pst_r)
                nc.vector.tensor_tensor(
                    v2lo_r[b][pi][:, j, :, 0:HD], pst_r, hi, op=SUB)

        def emit_attn(b, h, qh, fillers=()):
            # fillers: [(slot, thunk)]; popped at kb >= slot, so a filler's
            # products may only be consumed at kb >= slot (or later groups)
            fillers = sorted([e if isinstance(e, tuple) else (0, e)
                              for e in fillers], key=lambda e: e[0])
            pb = h * 64
            ps_o = [pso.tile([VW, 512], F32, tag="pso",
                             name=f"ps_o{q2}") for q2 in range(2)]

            def emit_av(pi, exv):
                for q2 in range(2):
                    nc.tensor.matmul(
                        ps_o[q2],
                        v2hi_r[b][pi][:, :, h, 0:VW],
                        exv[:, :, q2 * 512:(q2 + 1) * 512],
                        start=(pi == 0), stop=False, perf_mode=DR)
                    nc.tensor.matmul(
                        ps_o[q2],
                        v2lo_r[b][pi][:, :, h, 0:VW],
                        exv[:, :, q2 * 512:(q2 + 1) * 512],
                        start=False, stop=(pi == NPAIR - 1), perf_mode=DR)

            # attn@V for pair pi is emitted one pair late so PE never
            # hard-waits on the pair's last exp
            pend = None
            for pi in range(NPAIR):
                ex = pexp.tile([128, 2 * 1024], FP8, tag="expT",
                               name="expT")
                exv = ex.rearrange("p (j q) -> p j q", j=2)
                exu = ex.bitcast(U8).rearrange("p (j q) -> p j q", j=2)
                for j in range(2):
                    kb = 2 * pi + j
                    while fillers and fillers[0][0] <= kb:
                        fillers.pop(0)[1]()
                    ps_s = pbig.tile([128, 1024], F32, tag="scores",
                                     name="ps_s")
                    for q2 in range(2):
                        qc = qh * 2 + q2
                        nc.tensor.matmul(
                            ps_s[:, q2 * 512:(q2 + 1) * 512],
                            qk_sb[b][1][kb // 4][pb:pb + 64,
                                                 (kb % 4) * 128:
                                                 (kb % 4 + 1) * 128],
                            qk_sb[b][0][qc][pb:pb + 64, :],
                            start=True, stop=True)
                    if EXP_SCHED[kb] == "a":
                        nc.scalar.activation(exv[:, j, :], ps_s, Exp)
                    else:
                        nc.vector.tensor_scalar(
                            exu[:, j, :], ps_s, SCH_A, SCH_B,
                            op0=MUL, op1=ADD)
                    if j == 1 and pend is not None:
                        emit_av(*pend)
                pend = (pi, exv)
            emit_av(*pend)
            for _, f in fillers:
                f()
            # normalization head: recip + otmp now; bc+mult+ship deferred
            rec_s = pwork.tile([VW, 1024], F32R, tag="rec", name="rec_s")
            with nc.allow_low_precision(
                    reason="softmax denom recip rounded to f32r "
                           "for the PE broadcast"):
                for q2 in range(2):
                    nc.vector.reciprocal(
                        rec_s[HD:VW, q2 * 512:(q2 + 1) * 512],
                        ps_o[q2][HD:VW, :])
            otmps = []
            for q2 in range(2):
                otmp = pwork.tile([HD, 512], F32, tag="otmp",
                                  name="otmp")
                nc.scalar.copy(otmp, ps_o[q2][0:HD, :])
                otmps.append(otmp)

            def finish(q2):
                qc = qh * 2 + q2
                bc_ps = pps.tile([HD, 512], F32, tag="ps", name="bc_ps")
                nc.tensor.matmul(
                    bc_ps,
                    ones32[HD:VW, :],
                    rec_s[HD:VW, q2 * 512:(q2 + 1) * 512],
                    start=True, stop=True)
                nc.vector.tensor_tensor(
                    att_sb[b][h][:, qc * 512:(qc + 1) * 512],
                    otmps[q2], bc_ps[:, :], op=MUL)
                if qh == 1 and q2 == 1:
                    # (b,h) complete: ship the whole row in one DMA
                    nc.sync.dma_start(
                        out=a2a_in[h][b * 4:(b + 1) * 4, :, :].rearrange(
                            "j p s -> p j s"),
                        in_=att_sb[b][h].rearrange("p (j s) -> p j s", j=4))

            return [lambda: finish(0), lambda: finish(1)]

        srcb = a2a_out if with_collective else a2a_in

        def emit_unload(hi):
            for jj in range(4):
                eng = nc.sync if jj % 2 == 0 else nc.gpsimd
                eng.dma_start(
                    out=ao_sb[4 * hi + jj],
                    in_=srcb[hi][2 * jj:2 * jj + 2, :, :].rearrange(
                        "j p s -> (j p) s"))

        # output projection tile t = sm*2 + en (bf16)
        def emit_out1(t):
            sm, en = t // 2, t % 2
            ps = pps.tile([128, 512], F32, tag="ps", name="ps_out1")
            for kk in range(4):
                nc.tensor.matmul(
                    ps, ao_sb[kk][:, sm * 128:(sm + 1) * 128],
                    wout_sb[:, kk * D + en * 512:kk * D + (en + 1) * 512],
                    start=(kk == 0), stop=(kk == 3))
            nc.vector.scalar_tensor_tensor(
                part_sb[t], ps, 1.0 / WSC,
                bob_sb[:, en * 512:(en + 1) * 512], op0=MUL, op1=ADD)

        def emit_out2(t):
            sm, en = t // 2, t % 2
            ps = pps.tile([128, 512], F32, tag="ps", name="ps_out2")
            for kk in range(4, 8):
                nc.tensor.matmul(
                    ps, ao_sb[kk][:, sm * 128:(sm + 1) * 128],
                    wout_sb[:, kk * D + en * 512:kk * D + (en + 1) * 512],
                    start=(kk == 4), stop=(kk == 7))
            osb = pwork.tile([128, 512], F32, tag="outsb", name="osb")
            nc.vector.scalar_tensor_tensor(
                osb, ps, 1.0 / WSC, part_sb[t], op0=MUL, op1=ADD)
            eng = nc.sync if t % 2 == 0 else nc.gpsimd
            eng.dma_start(
                out=out[sm * 128:(sm + 1) * 128, en * 512:(en + 1) * 512],
                in_=osb)

        def F(fn, *a):
            return lambda: fn(*a)

        # Filler safety rule: a filler popped at kb-slot i is emitted just
        # before slot i's scores, so anything it produces may only be
        # consumed at kb >= i (or by a later group).
        emit_qk(0, 1, 0)
        emit_qk(0, 1, 1)
        emit_v_mm(0, 0)
        emit_qk(0, 0, 0)
        emit_v_tr(0, 0)
        emit_qk(0, 0, 1)
        emit_v_mm(0, 1)
        fin = emit_attn(0, 0, 0, fillers=(
            [(0, F(emit_v_tr, 0, 1)),
             (2, F(emit_v_mm, 0, 2)), (4, F(emit_v_tr, 0, 2)),
             (6, F(emit_qk, 0, 1, 2)),
             (8, F(emit_v_mm, 0, 3)), (10, F(emit_v_tr, 0, 3)),
             (11, F(emit_qk, 0, 1, 3)),
             (13, F(emit_qk, 0, 0, 2)), (15, F(emit_qk, 0, 0, 3))]))
        fin = emit_attn(0, 1, 0, fillers=[(1, fin[0]), (3, fin[1])])
        fin = emit_attn(0, 0, 1, fillers=(
            [(1, fin[0]), (3, fin[1])]
            + [(i * 2, F(emit_qk, 1, 1, qn))
               for i, qn in enumerate(range(NQC))]
            + [(8, F(emit_qk, 1, 0, 0)), (10, F(emit_qk, 1, 0, 1))]))
        fin = emit_attn(0, 1, 1, fillers=(
            [(1, fin[0]), (3, fin[1])]
            + [(0, F(emit_v_mm, 1, 0)), (2, F(emit_v_tr, 1, 0)),
               (4, F(emit_v_mm, 1, 1)), (6, F(emit_v_tr, 1, 1))]
            + [(14, F(emit_qk, 1, 0, 2)), (15, F(emit_qk, 1, 0, 3))]))
        fin = emit_attn(1, 0, 0, fillers=(
            [(1, fin[0]), (3, fin[1])]
            + [(0, F(emit_v_mm, 1, 2)), (2, F(emit_v_tr, 1, 2)),
               (4, F(emit_v_mm, 1, 3)), (6, F(emit_v_tr, 1, 3))]))
        fin = emit_attn(1, 0, 1, fillers=[(1, fin[0]), (3, fin[1])])
        # (1,1,0) flushes the last even-head finishers -> a2a #0 can go
        fin = emit_attn(1, 1, 0, fillers=[(1, fin[0]), (3, fin[1])])
        emit_a2a(0)
        emit_unload(0)
        fin2 = emit_attn(1, 1, 1, fillers=(
            [(1, fin[0]), (3, fin[1])]
            + [(2 * t, F(emit_out1, t)) for t in range(8)]))
        for f in fin2:
            f()
        emit_a2a(1)
        emit_unload(1)
        for t in range(8):
            emit_out2(t)

    nc.compile()
    return nc


_NC_CACHE = {}


def _get_nc(with_collective: bool = True):
    key = bool(with_collective)
    if key not in _NC_CACHE:
        _NC_CACHE[key] = _build_nc(with_collective)
    return _NC_CACHE[key]


def make_in_maps(x, w_qkv, b_qkv, w_out, b_out):
    """Host-side sharding/prep. Returns per-core input dicts."""
    x = np.asarray(x, dtype=np.float32)
    w_qkv = np.asarray(w_qkv, dtype=np.float32)
    b_qkv = np.asarray(b_qkv, dtype=np.float32)
    w_out = np.asarray(w_out, dtype=np.float32)
    b_out = np.asarray(b_out, dtype=np.float32)

    wq = w_qkv[0:D].reshape(H, HD, D)
    wk = w_qkv[D:2 * D].reshape(H, HD, D)
    wv = w_qkv[2 * D:3 * D].reshape(H, HD, D)
    bq = b_qkv[0:D].reshape(H, HD)
    bk = b_qkv[D:2 * D].reshape(H, HD)
    scale = 1.0 / np.sqrt(HD)

    perm = np.concatenate(
        [np.arange(h * HD, (h + 1) * HD) for h in range(0, H, 2)]
        + [np.arange(h * HD, (h + 1) * HD) for h in range(1, H, 2)])
    wout_t = np.ascontiguousarray(w_out.T[perm]).astype(NPBF16)
    # v bias passes through attention unchanged (softmax rows sum to 1),
    # so its out-proj image folds into the output bias
    bv_full = b_qkv[2 * D:3 * D]
    bob_vec = b_out + w_out @ bv_full
    bob = np.ascontiguousarray(
        np.broadcast_to(bob_vec, (128, D))).astype(np.float32)

    # [d, 4096] stacked batch-major; fp8 hi + fp8 residual
    xt_f32 = np.ascontiguousarray(
        np.concatenate([x[0].T, x[1].T], axis=1))
    xt_all = xt_f32.astype(NPFP8)
    xres_all = (xt_f32 - xt_all.astype(np.float32)).astype(NPFP8)

    in_maps = []
    for c in range(NCORES):
        hs = slice(c * HPC, (c + 1) * HPC)
        wq_c = (wq[hs].reshape(FPC, D) * WSC).T
        wk_c = (wk[hs].reshape(FPC, D) * WSC).T
        wqk_c = np.concatenate([wq_c, wk_c], axis=1).astype(NPFP8)
        bqk_c = np.concatenate([bq[hs].reshape(FPC) * scale,
                                bk[hs].reshape(FPC)])
        bqk_c = np.ascontiguousarray(
            bqk_c.reshape(2, 128).T).astype(np.float32)
        wv_f = (wv[hs].reshape(FPC, D) * WSC).T
        wv_c = wv_f.astype(NPFP8)
        wvr_c = (wv_f - wv_c.astype(np.float32)).astype(NPFP8)
        in_maps.append({
            "onesr": np.ones((VW, HD), dtype=np.float32),
            "ident": np.eye(128, dtype=np.float32).astype(NPBF16),
            "xt": xt_all,
            "xres": xres_all,
            "wqk": np.ascontiguousarray(wqk_c),
            "bqk": bqk_c,
            "wv": np.ascontiguousarray(wv_c),
            "wvr": np.ascontiguousarray(wvr_c),
            "wout": wout_t,
            "bob": bob,
        })
    return in_maps


def assemble_output(results):
    out = np.empty((B, S, D), dtype=np.float32)
    for c in range(NCORES):
        b, sg = c // 4, c % 4
        out[b, sg * SC:(sg + 1) * SC, :] = results[c]["out"]
    return out


def kernel(x, mask, w_qkv, b_qkv, w_out, b_out):
    nc = _get_nc(True)
    in_maps = make_in_maps(x, w_qkv, b_qkv, w_out, b_out)
    res = run_bass_kernel_spmd(nc, in_maps, core_ids=list(range(NCORES)))
    return assemble_output(res.results)


# revision 44
# speedup vs baseline: 1.0106x; 1.0030x over previous
"""Multi-head attention on 8 Trainium2 NeuronCores (Bass/Tile), fp8 edition.

Problem: x[2,2048,1024] -> qkv proj (16 heads, hd=64) -> softmax(QK^T/8)V
-> out proj.  mask is all-ones (per spec) and is ignored.

Sharding: core c owns heads {2c, 2c+1} for BOTH batches (tensor-parallel
QKV + attention).  An 8-core AllToAll converts the head-sharded attention
output into a sequence-sharded full-feature activation; core c ends up
with global row chunk c (batch c//4, rows (c%4)*512..) and computes the
output projection full-width with no all-reduce.

Precision/speed strategy (cost model: fp8e4 DoubleRow matmul = 0.5
cycles/row, i.e. 4x bf16 per FLOP at K-pair packing):
  - qk projection: fp8 DoubleRow, 2-term (x8@w8 + xres8@w8) to keep
    q/k noise small; score-side noise averages through softmax.
  - v projection: fp8 DoubleRow, 3-term (x8@wv8 + x8@wvres + xres8@wv8).
  - scores: bf16 (hd=64 contraction can't DoubleRow without expensive
    partition-split copies).
  - exp(scores)->fp8 probabilities on TWO engines: ACT (native Exp, fp8
    out) and DVE (Schraudolph: u8 = round(x*8/ln2 + 55.63) IS the e4m3
    encoding of ~e^x).  GPSIMD can't read PSUM, so it gets no exp work.
  - attn@V: fp8 DoubleRow over key-block pairs, with V stored hi+lo
    (v2hi = fp8(32v), v2lo = fp8(32v - v2hi)) accumulating into the SAME
    PSUM group -> v quantization error cancels.  A ones column in v2hi
    (zeros in v2lo) accumulates the softmax denominator in PSUM row 64.
  - att = 32*num/den in bf16; out projection in plain bf16.
  - v bias and out bias fold into bob = b_out + w_out@bv host-side.

Schedule: normalization finishers (bc matmul + mult + ship chunk) of
each attention group are DEFERRED into the next group's filler slots so
PE never stalls on the recip; output DMAs are split into 128KB chunks
to shrink the serial tail.

PSUM: scores [128,1024] x2 (4 banks) + pso [65,512] x2 (2) + pps
[128,512] x2 (2, also hosts bc broadcasts) = 8 banks.
"""

import numpy as np
import ml_dtypes
from contextlib import ExitStack

import concourse.bass as bass
import concourse.mybir as mybir
import concourse.tile as tile
from concourse import bacc
from concourse.bass_utils import run_bass_kernel_spmd

BF16 = mybir.dt.bfloat16
F32 = mybir.dt.float32
F32R = mybir.dt.float32r
FP8 = mybir.dt.float8e4
U8 = mybir.dt.uint8
NPBF16 = ml_dtypes.bfloat16
NPFP8 = ml_dtypes.float8_e4m3

D, H, HD, B, S = 1024, 16, 64, 2, 2048
NCORES = 8
HPC = 2              # heads per core
FPC = HPC * HD       # 128 features per core
SS = B * S           # 4096 stacked sequence (batch-major)
SC = 512             # output rows per core (post all-to-all)
NKB = S // 128       # 16 key blocks per batch
NPAIR = NKB // 2     # 8 key-block pairs (DoubleRow)
NQC = S // 512       # 4 query chunks per batch
VW = HD + 1          # v columns per head incl. ones column
VP = 80              # padded per-head stride in v2 tiles
WSC = 32.0           # fp8 weight scale

Exp = mybir.ActivationFunctionType.Exp
DR = mybir.MatmulPerfMode.DoubleRow
MUL = mybir.AluOpType.mult
ADD = mybir.AluOpType.add
SUB = mybir.AluOpType.subtract

# Schraudolph exp->e4m3: bits = round(x * 8/ln2 + SCH_B)
SCH_A = 8.0 / np.log(2.0)
SCH_B = 55.63

# per-group exp engine schedule (16 key blocks): a=ACT, d=DVE
EXP_SCHED = "adaadaadaadaadaa"


def _build_nc(with_collective: bool = True):
    nc = bacc.Bacc("TRN2", target_bir_lowering=False, debug=False,
                   num_devices=NCORES)
    xt = nc.dram_tensor("xt", [D, SS], FP8, kind="ExternalInput").ap()
    xres = nc.dram_tensor("xres", [D, SS], FP8, kind="ExternalInput").ap()
    wqk = nc.dram_tensor("wqk", [D, 2 * FPC], FP8, kind="ExternalInput").ap()
    bqk = nc.dram_tensor("bqk", [128, 2], F32, kind="ExternalInput").ap()
    wv = nc.dram_tensor("wv", [D, FPC], FP8, kind="ExternalInput").ap()
    wvr = nc.dram_tensor("wvr", [D, FPC], FP8, kind="ExternalInput").ap()
    wout = nc.dram_tensor("wout", [D, D], BF16, kind="ExternalInput").ap()
    bob = nc.dram_tensor("bob", [128, D], F32, kind="ExternalInput").ap()
    ident = nc.dram_tensor("ident", [128, 128], BF16,
                           kind="ExternalInput").ap()
    onesr = nc.dram_tensor("onesr", [VW, HD], F32R,
                           kind="ExternalInput").ap()
    out = nc.dram_tensor("out", [SC, D], F32, kind="ExternalOutput").ap()

    with ExitStack() as ctx:
        tc = ctx.enter_context(tile.TileContext(nc))
        persist = ctx.enter_context(tc.tile_pool(name="persist", bufs=1))
        pexp = ctx.enter_context(tc.tile_pool(name="pexp", bufs=4))
        pwork = ctx.enter_context(tc.tile_pool(name="pwork", bufs=4))
        pbig = ctx.enter_context(tc.tile_pool(name="pbig", bufs=2, space="PSUM"))
        pso = ctx.enter_context(tc.tile_pool(name="pso", bufs=2, space="PSUM"))
        pps = ctx.enter_context(tc.tile_pool(name="pps", bufs=2, space="PSUM"))
        dram = ctx.enter_context(tc.tile_pool(name="dram", bufs=2, space="DRAM"))

        # ---------------- persistent SBUF ----------------
        # xt_sb[b]: [128, (i 8, s 2048)] fp8; xr_sb = fp8 residual
        xt_sb = [persist.tile([128, 8 * S], FP8, tag=f"xt{b}",
                              name=f"xt{b}") for b in range(B)]
        xt_r = [t.rearrange("p (i s) -> p i s", i=8) for t in xt_sb]
        xr_sb = [persist.tile([128, 8 * S], FP8, tag=f"xr{b}",
                              name=f"xr{b}") for b in range(B)]
        xr_r = [t.rearrange("p (i s) -> p i s", i=8) for t in xr_sb]
        wqk_sb = persist.tile([128, 8 * 2 * FPC], FP8, tag="wqk",
                              name="wqk_sb")
        wqk_r = wqk_sb.rearrange("p (i f) -> p i f", i=8)
        bqk_sb = persist.tile([128, 2], F32, tag="bqk", name="bqk")
        wv_sb = persist.tile([128, 8 * FPC], FP8, tag="wv", name="wv_sb")
        wv_r = wv_sb.rearrange("p (i f) -> p i f", i=8)
        wvr_sb = persist.tile([128, 8 * FPC], FP8, tag="wvr", name="wvr_sb")
        wvr_r = wvr_sb.rearrange("p (i f) -> p i f", i=8)
        wout_sb = persist.tile([128, 8 * D], BF16, tag="wout", name="wout_sb")
        wout_r = wout_sb.rearrange("p (i f) -> p i f", i=8)
        bob_sb = persist.tile([128, D], F32, tag="bob", name="bob")
        ident_sb = persist.tile([128, 128], BF16, tag="ident", name="ident")
        qk_sb = [[[persist.tile([128, 512], BF16, tag=f"qk{b}_{m}_{qn}",
                                name=f"qk{b}_{m}_{qn}") for qn in range(NQC)]
                  for m in range(2)]
                 for b in range(B)]
        # v2{hi,lo}[b][pi]: [128, 2*(2*VP)] fp8 - key-block pair pi; layout
        # [p, j(2), h(2), c(VP)], v at c 0:64; ones col 64: 1.0 in hi, 0 in lo
        v2hi = [[persist.tile([128, 2 * 2 * VP], FP8, tag=f"vh{b}_{pi}",
                              name=f"vh{b}_{pi}") for pi in range(NPAIR)]
                for b in range(B)]
        v2lo = [[persist.tile([128, 2 * 2 * VP], FP8, tag=f"vl{b}_{pi}",
                              name=f"vl{b}_{pi}") for pi in range(NPAIR)]
                for b in range(B)]
        v2hi_r = [[t.rearrange("p (j h c) -> p j h c", j=2, h=2)
                   for t in row] for row in v2hi]
        v2lo_r = [[t.rearrange("p (j h c) -> p j h c", j=2, h=2)
                   for t in row] for row in v2lo]
        att_sb = [[persist.tile([64, S], BF16, tag=f"att{b}_{h}",
                                name=f"att{b}_{h}") for h in range(HPC)]
                  for b in range(B)]
        ones32 = persist.tile([VW, HD], F32R, tag="ones32", name="ones32")
        nc.sync.dma_start(out=ones32, in_=onesr[:, :])
        ao_sb = [persist.tile([128, SC], BF16, tag=f"ao{i}", name=f"ao{i}")
                 for i in range(8)]
        part_sb = [persist.tile([128, 512], F32, tag=f"part{t}",
                                name=f"part{t}") for t in range(8)]
        for b in range(B):
            for pi in range(NPAIR):
                nc.gpsimd.memset(v2hi_r[b][pi][:, :, :, HD:VW], 1.0)
                nc.gpsimd.memset(v2lo_r[b][pi][:, :, :, HD:VW], 0.0)

        # ---------------- loads (ordered by first use) ----------------
        def load_x(dst_r, src, b, s0, s1, eng=None):
            # (p, i, s) -> src[i*128 + p, b*S + s]
            ap = bass.AP(tensor=src.tensor, offset=b * S + s0,
                         ap=[[SS, 128], [128 * SS, 8], [1, s1 - s0]])
            (eng or nc.sync).dma_start(out=dst_r[b][:, :, s0:s1], in_=ap)

        wqk_src = bass.AP(tensor=wqk.tensor, offset=0,
                          ap=[[2 * FPC, 128], [128 * 2 * FPC, 8], [1, 2 * FPC]])
        nc.sync.dma_start(out=wqk_r, in_=wqk_src)
        load_x(xt_r, xt, 0, 0, 512)
        load_x(xr_r, xres, 0, 0, 512)
        nc.sync.dma_start(out=bqk_sb, in_=bqk[:, :])
        load_x(xt_r, xt, 0, 512, 2048)
        load_x(xr_r, xres, 0, 512, 2048)
        wv_src = bass.AP(tensor=wv.tensor, offset=0,
                         ap=[[FPC, 128], [128 * FPC, 8], [1, FPC]])
        nc.sync.dma_start(out=wv_r, in_=wv_src)
        wvr_src = bass.AP(tensor=wvr.tensor, offset=0,
                          ap=[[FPC, 128], [128 * FPC, 8], [1, FPC]])
        nc.sync.dma_start(out=wvr_r, in_=wvr_src)
        nc.sync.dma_start(out=ident_sb, in_=ident[:, :])
        load_x(xt_r, xt, 1, 0, 2048)
        load_x(xr_r, xres, 1, 0, 2048)
        wout_src = bass.AP(tensor=wout.tensor, offset=0,
                           ap=[[D, 128], [128 * D, 8], [1, D]])
        nc.sync.dma_start(out=wout_r, in_=wout_src)
        nc.sync.dma_start(out=bob_sb, in_=bob[:, :])

        a2a_in = [dram.tile([8, HD, SC], BF16, tag=f"a2a_in{h}",
                            name=f"a2a_in{h}", bufs=1) for h in range(HPC)]
        a2a_out = [dram.tile([8, HD, SC], BF16, tag=f"a2a_out{h}",
                             name=f"a2a_out{h}", bufs=1) for h in range(HPC)]

        def emit_a2a(h):
            if with_collective:
                nc.gpsimd.collective_compute(
                    "AllToAll", mybir.AluOpType.bypass,
                    replica_groups=[list(range(8))],
                    ins=[a2a_in[h][:, :, :].opt()],
                    outs=[a2a_out[h][:, :, :].opt()])

        # ------------- projections + attention, interleaved -------------
        def emit_qk(b, m, qn):
            ps = pps.tile([128, 512], F32, tag="ps", name="ps_qk")
            for i in range(4):
                nc.tensor.matmul(
                    ps,
                    wqk_r[:, 2 * i:2 * i + 2, m * 128:(m + 1) * 128],
                    xt_r[b][:, 2 * i:2 * i + 2, qn * 512:(qn + 1) * 512],
                    start=(i == 0), stop=False, perf_mode=DR)
            for i in range(4):
                nc.tensor.matmul(
                    ps,
                    wqk_r[:, 2 * i:2 * i + 2, m * 128:(m + 1) * 128],
                    xr_r[b][:, 2 * i:2 * i + 2, qn * 512:(qn + 1) * 512],
                    start=False, stop=(i == 3), perf_mode=DR)
            nc.vector.tensor_scalar(
                qk_sb[b][m][qn], ps,
                (0.125 / WSC) if m == 0 else (1.0 / WSC),
                bqk_sb[:, m:m + 1], op0=MUL, op1=ADD)

        # v projection, operand-swapped: out [128 feat, 512 seq] in one
        # 12-matmul pass per (b, cq); transposed back per 128-seq block
        vfm_store = {}

        def emit_v_mm(b, cq):
            ps = pps.tile([128, 512], F32, tag="ps", name="ps_v")
            terms = [(wv_r, xt_r), (wvr_r, xt_r), (wv_r, xr_r)]
            n = 0
            for wsrc, xsrc in terms:
                for i in range(4):
                    nc.tensor.matmul(
                        ps,
                        wsrc[:, 2 * i:2 * i + 2, :],
                        xsrc[b][:, 2 * i:2 * i + 2,
                                cq * 512:(cq + 1) * 512],
                        start=(n == 0), stop=(n == 11), perf_mode=DR)
                    n += 1
            vfm = pwork.tile([128, 512], BF16, tag="vfm", name="vfm")
            if (b * 4 + cq) % 2:
                nc.scalar.copy(vfm, ps)
            else:
                nc.vector.tensor_copy(vfm, ps)
            vfm_store[(b, cq)] = vfm

        def emit_v_tr(b, cq):
            vfm = vfm_store.pop((b, cq))
            for s4 in range(4):
                sn = cq * 4 + s4
                pi, j = sn // 2, sn % 2
                pst = pps.tile([128, 128], BF16, tag="ps", name="pst")
                nc.tensor.transpose(
                    pst, vfm[:, s4 * 128:(s4 + 1) * 128], ident_sb)
                pst_r = pst.rearrange("p (h c) -> p h c", h=2)
                hi = v2hi_r[b][pi][:, j, :, 0:HD]
                nc.vector.tensor_copy(hi, pst_r)
                nc.vector.tensor_tensor(
                    v2lo_r[b][pi][:, j, :, 0:HD], pst_r, hi, op=SUB)

        def emit_attn(b, h, qh, fillers=()):
            # fillers: [(slot, thunk)]; popped at kb >= slot, so a filler's
            # products may only be consumed at kb >= slot (or later groups)
            fillers = sorted([e if isinstance(e, tuple) else (0, e)
                              for e in fillers], key=lambda e: e[0])
            pb = h * 64
            ps_o = [pso.tile([VW, 512], F32, tag="pso",
                             name=f"ps_o{q2}") for q2 in range(2)]

            def emit_av(pi, exv):
                for q2 in range(2):
                    nc.tensor.matmul(
                        ps_o[q2],
                        v2hi_r[b][pi][:, :, h, 0:VW],
                        exv[:, :, q2 * 512:(q2 + 1) * 512],
                        start=(pi == 0), stop=False, perf_mode=DR)
                    nc.tensor.matmul(
                        ps_o[q2],
                        v2lo_r[b][pi][:, :, h, 0:VW],
                        exv[:, :, q2 * 512:(q2 + 1) * 512],
                        start=False, stop=(pi == NPAIR - 1), perf_mode=DR)

            # attn@V for pair pi is emitted one pair late so PE never
            # hard-waits on the pair's last exp
            pend = None
            for pi in range(NPAIR):
                ex = pexp.tile([128, 2 * 1024], FP8, tag="expT",
                               name="expT")
                exv = ex.rearrange("p (j q) -> p j q", j=2)
                exu = ex.bitcast(U8).rearrange("p (j q) -> p j q", j=2)
                for j in range(2):
                    kb = 2 * pi + j
                    while fillers and fillers[0][0] <= kb:
                        fillers.pop(0)[1]()
                    ps_s = pbig.tile([128, 1024], F32, tag="scores",
                                     name="ps_s")
                    for q2 in range(2):
                        qc = qh * 2 + q2
                        nc.tensor.matmul(
                            ps_s[:, q2 * 512:(q2 + 1) * 512],
                            qk_sb[b][1][kb // 4][pb:pb + 64,
                                                 (kb % 4) * 128:
                                                 (kb % 4 + 1) * 128],
                            qk_sb[b][0][qc][pb:pb + 64, :],
                            start=True, stop=True)
                    if EXP_SCHED[kb] == "a":
                        nc.scalar.activation(exv[:, j, :], ps_s, Exp)
                    else:
                        nc.vector.tensor_scalar(
                            exu[:, j, :], ps_s, SCH_A, SCH_B,
                            op0=MUL, op1=ADD)
                    if j == 1 and pend is not None:
                        emit_av(*pend)
                pend = (pi, exv)
            emit_av(*pend)
            for _, f in fillers:
                f()
            # normalization head: recip + otmp now; bc+mult+ship deferred
            rec_s = pwork.tile([VW, 1024], F32R, tag="rec", name="rec_s")
            with nc.allow_low_precision(
                    reason="softmax denom recip rounded to f32r "
                           "for the PE broadcast"):
                for q2 in range(2):
                    nc.vector.reciprocal(
                        rec_s[HD:VW, q2 * 512:(q2 + 1) * 512],
                        ps_o[q2][HD:VW, :])
            otmps = []
            for q2 in range(2):
                otmp = pwork.tile([HD, 512], F32, tag="otmp",
                                  name="otmp")
                nc.scalar.copy(otmp, ps_o[q2][0:HD, :])
                otmps.append(otmp)

            def finish(q2):
                qc = qh * 2 + q2
                bc_ps = pps.tile([HD, 512], F32, tag="ps", name="bc_ps")
                nc.tensor.matmul(
                    bc_ps,
                    ones32[HD:VW, :],
                    rec_s[HD:VW, q2 * 512:(q2 + 1) * 512],
                    start=True, stop=True)
                nc.vector.tensor_tensor(
                    att_sb[b][h][:, qc * 512:(qc + 1) * 512],
                    otmps[q2], bc_ps[:, :], op=MUL)
                if qh == 1 and q2 == 1:
                    # (b,h) complete: ship the whole row in one DMA
                    nc.sync.dma_start(
                        out=a2a_in[h][b * 4:(b + 1) * 4, :, :].rearrange(
                            "j p s -> p j s"),
                        in_=att_sb[b][h].rearrange("p (j s) -> p j s", j=4))

            return [lambda: finish(0), lambda: finish(1)]

        srcb = a2a_out if with_collective else a2a_in

        def emit_unload(hi):
            for jj in range(4):
                nc.sync.dma_start(
                    out=ao_sb[4 * hi + jj],
                    in_=srcb[hi][2 * jj:2 * jj + 2, :, :].rearrange(
                        "j p s -> (j p) s"))

        # output projection tile t = sm*2 + en (bf16)
        def emit_out1(t):
            sm, en = t // 2, t % 2
            ps = pps.tile([128, 512], F32, tag="ps", name="ps_out1")
            for kk in range(4):
                nc.tensor.matmul(
                    ps, ao_sb[kk][:, sm * 128:(sm + 1) * 128],
                    wout_sb[:, kk * D + en * 512:kk * D + (en + 1) * 512],
                    start=(kk == 0), stop=(kk == 3))
            nc.vector.scalar_tensor_tensor(
                part_sb[t], ps, 1.0 / WSC,
                bob_sb[:, en * 512:(en + 1) * 512], op0=MUL, op1=ADD)

        def emit_out2(t):
            sm, en = t // 2, t % 2
            ps = pps.tile([128, 512], F32, tag="ps", name="ps_out2")
            for kk in range(4, 8):
                nc.tensor.matmul(
                    ps, ao_sb[kk][:, sm * 128:(sm + 1) * 128],
                    wout_sb[:, kk * D + en * 512:kk * D + (en + 1) * 512],
                    start=(kk == 4), stop=(kk == 7))
            osb = pwork.tile([128, 512], F32, tag="outsb", name="osb")
            nc.vector.scalar_tensor_tensor(
                osb, ps, 1.0 / WSC, part_sb[t], op0=MUL, op1=ADD)
            nc.sync.dma_start(
                out=out[sm * 128:(sm + 1) * 128, en * 512:(en + 1) * 512],
                in_=osb)

        def F(fn, *a):
            return lambda: fn(*a)

        # Filler safety rule: a filler popped at kb-slot i is emitted just
        # before slot i's scores, so anything it produces may only be
        # consumed at kb >= i (or by a later group).
        emit_qk(0, 1, 0)
        emit_qk(0, 1, 1)
        emit_v_mm(0, 0)
        emit_qk(0, 0, 0)
        emit_v_tr(0, 0)
        emit_qk(0, 0, 1)
        emit_v_mm(0, 1)
        fin = emit_attn(0, 0, 0, fillers=(
            [(0, F(emit_v_tr, 0, 1)),
             (2, F(emit_v_mm, 0, 2)), (4, F(emit_v_tr, 0, 2)),
             (6, F(emit_qk, 0, 1, 2)),
             (8, F(emit_v_mm, 0, 3)), (10, F(emit_v_tr, 0, 3)),
             (11, F(emit_qk, 0, 1, 3)),
             (13, F(emit_qk, 0, 0, 2)), (15, F(emit_qk, 0, 0, 3))]))
        fin = emit_attn(0, 1, 0, fillers=[(1, fin[0]), (3, fin[1])])
        fin = emit_attn(0, 0, 1, fillers=(
            [(1, fin[0]), (3, fin[1])]
            + [(i * 2, F(emit_qk, 1, 1, qn))
               for i, qn in enumerate(range(NQC))]
            + [(8, F(emit_qk, 1, 0, 0)), (10, F(emit_qk, 1, 0, 1))]))
        fin = emit_attn(0, 1, 1, fillers=(
            [(1, fin[0]), (3, fin[1])]
            + [(0, F(emit_v_mm, 1, 0)), (2, F(emit_v_tr, 1, 0)),
               (4, F(emit_v_mm, 1, 1)), (6, F(emit_v_tr, 1, 1))]
            + [(14, F(emit_qk, 1, 0, 2)), (15, F(emit_qk, 1, 0, 3))]))
        fin = emit_attn(1, 0, 0, fillers=(
            [(1, fin[0]), (3, fin[1])]
            + [(0, F(emit_v_mm, 1, 2)), (2, F(emit_v_tr, 1, 2)),
               (4, F(emit_v_mm, 1, 3)), (6, F(emit_v_tr, 1, 3))]))
        fin = emit_attn(1, 0, 1, fillers=[(1, fin[0]), (3, fin[1])])
        # (1,1,0) flushes the last even-head finishers -> a2a #0 can go
        fin = emit_attn(1, 1, 0, fillers=[(1, fin[0]), (3, fin[1])])
        emit_a2a(0)
        emit_unload(0)
        fin2 = emit_attn(1, 1, 1, fillers=(
            [(1, fin[0]), (3, fin[1])]
            + [(2 * t, F(emit_out1, t)) for t in range(8)]))
        for f in fin2:
            f()
        emit_a2a(1)
        emit_unload(1)
        for t in range(8):
            emit_out2(t)

    nc.compile()
    return nc


_NC_CACHE = {}


def _get_nc(with_collective: bool = True):
    key = bool(with_collective)
    if key not in _NC_CACHE:
        _NC_CACHE[key] = _build_nc(with_collective)
    return _NC_CACHE[key]


def make_in_maps(x, w_qkv, b_qkv, w_out, b_out):
    """Host-side sharding/prep. Returns per-core input dicts."""
    x = np.asarray(x, dtype=np.float32)
    w_qkv = np.asarray(w_qkv, dtype=np.float32)
    b_qkv = np.asarray(b_qkv, dtype=np.float32)
    w_out = np.asarray(w_out, dtype=np.float32)
    b_out = np.asarray(b_out, dtype=np.float32)

    wq = w_qkv[0:D].reshape(H, HD, D)
    wk = w_qkv[D:2 * D].reshape(H, HD, D)
    wv = w_qkv[2 * D:3 * D].reshape(H, HD, D)
    bq = b_qkv[0:D].reshape(H, HD)
    bk = b_qkv[D:2 * D].reshape(H, HD)
    scale = 1.0 / np.sqrt(HD)

    perm = np.concatenate(
        [np.arange(h * HD, (h + 1) * HD) for h in range(0, H, 2)]
        + [np.arange(h * HD, (h + 1) * HD) for h in range(1, H, 2)])
    wout_t = np.ascontiguousarray(w_out.T[perm]).astype(NPBF16)
    # v bias passes through attention unchanged (softmax rows sum to 1),
    # so its out-proj image folds into the output bias
    bv_full = b_qkv[2 * D:3 * D]
    bob_vec = b_out + w_out @ bv_full
    bob = np.ascontiguousarray(
        np.broadcast_to(bob_vec, (128, D))).astype(np.float32)

    # [d, 4096] stacked batch-major; fp8 hi + fp8 residual
    xt_f32 = np.ascontiguousarray(
        np.concatenate([x[0].T, x[1].T], axis=1))
    xt_all = xt_f32.astype(NPFP8)
    xres_all = (xt_f32 - xt_all.astype(np.float32)).astype(NPFP8)

    in_maps = []
    for c in range(NCORES):
        hs = slice(c * HPC, (c + 1) * HPC)
        wq_c = (wq[hs].reshape(FPC, D) * WSC).T
        wk_c = (wk[hs].reshape(FPC, D) * WSC).T
        wqk_c = np.concatenate([wq_c, wk_c], axis=1).astype(NPFP8)
        bqk_c = np.concatenate([bq[hs].reshape(FPC) * scale,
                                bk[hs].reshape(FPC)])
        bqk_c = np.ascontiguousarray(
            bqk_c.reshape(2, 128).T).astype(np.float32)
        wv_f = (wv[hs].reshape(FPC, D) * WSC).T
        wv_c = wv_f.astype(NPFP8)
        wvr_c = (wv_f - wv_c.astype(np.float32)).astype(NPFP8)
        in_maps.append({
            "onesr": np.ones((VW, HD), dtype=np.float32),
            "ident": np.eye(128, dtype=np.float32).astype(NPBF16),
            "xt": xt_all,
            "xres": xres_all,
            "wqk": np.ascontiguousarray(wqk_c),
            "bqk": bqk_c,
            "wv": np.ascontiguousarray(wv_c),
            "wvr": np.ascontiguousarray(wvr_c),
            "wout": wout_t,
            "bob": bob,
        })
    return in_maps


def assemble_output(results):
    out = np.empty((B, S, D), dtype=np.float32)
    for c in range(NCORES):
        b, sg = c // 4, c % 4
        out[b, sg * SC:(sg + 1) * SC, :] = results[c]["out"]
    return out


def kernel(x, mask, w_qkv, b_qkv, w_out, b_out):
    nc = _get_nc(True)
    in_maps = make_in_maps(x, w_qkv, b_qkv, w_out, b_out)
    res = run_bass_kernel_spmd(nc, in_maps, core_ids=list(range(NCORES)))
    return assemble_output(res.results)


# revision 45
# speedup vs baseline: 1.0208x; 1.0101x over previous
"""Multi-head attention on 8 Trainium2 NeuronCores (Bass/Tile), fp8 edition.

Problem: x[2,2048,1024] -> qkv proj (16 heads, hd=64) -> softmax(QK^T/8)V
-> out proj.  mask is all-ones (per spec) and is ignored.

Sharding: core c owns heads {2c, 2c+1} for BOTH batches (tensor-parallel
QKV + attention).  An 8-core AllToAll converts the head-sharded attention
output into a sequence-sharded full-feature activation; core c ends up
with global row chunk c (batch c//4, rows (c%4)*512..) and computes the
output projection full-width with no all-reduce.

Precision/speed strategy (cost model: fp8e4 DoubleRow matmul = 0.5
cycles/row, i.e. 4x bf16 per FLOP at K-pair packing):
  - qk projection: fp8 DoubleRow, 2-term (x8@w8 + xres8@w8) to keep
    q/k noise small; score-side noise averages through softmax.
  - v projection: fp8 DoubleRow, 3-term (x8@wv8 + x8@wvres + xres8@wv8).
  - scores: bf16 (hd=64 contraction can't DoubleRow without expensive
    partition-split copies).
  - exp(scores)->fp8 probabilities on TWO engines: ACT (native Exp, fp8
    out) and DVE (Schraudolph: u8 = round(x*8/ln2 + 55.63) IS the e4m3
    encoding of ~e^x).  GPSIMD can't read PSUM, so it gets no exp work.
  - attn@V: fp8 DoubleRow over key-block pairs, with V stored hi+lo
    (v2hi = fp8(32v), v2lo = fp8(32v - v2hi)) accumulating into the SAME
    PSUM group -> v quantization error cancels.  A ones column in v2hi
    (zeros in v2lo) accumulates the softmax denominator in PSUM row 64.
  - att = 32*num/den in bf16; out projection in plain bf16.
  - v bias and out bias fold into bob = b_out + w_out@bv host-side.

Schedule: normalization finishers (bc matmul + mult + ship chunk) of
each attention group are DEFERRED into the next group's filler slots so
PE never stalls on the recip; output DMAs are split into 128KB chunks
to shrink the serial tail.

PSUM: scores [128,1024] x2 (4 banks) + pso [65,512] x2 (2) + pps
[128,512] x2 (2, also hosts bc broadcasts) = 8 banks.
"""

import numpy as np
import ml_dtypes
from contextlib import ExitStack

import concourse.bass as bass
import concourse.mybir as mybir
import concourse.tile as tile
from concourse import bacc
from concourse.bass_utils import run_bass_kernel_spmd

BF16 = mybir.dt.bfloat16
F32 = mybir.dt.float32
F32R = mybir.dt.float32r
FP8 = mybir.dt.float8e4
U8 = mybir.dt.uint8
NPBF16 = ml_dtypes.bfloat16
NPFP8 = ml_dtypes.float8_e4m3

D, H, HD, B, S = 1024, 16, 64, 2, 2048
NCORES = 8
HPC = 2              # heads per core
FPC = HPC * HD       # 128 features per core
SS = B * S           # 4096 stacked sequence (batch-major)
SC = 512             # output rows per core (post all-to-all)
NKB = S // 128       # 16 key blocks per batch
NPAIR = NKB // 2     # 8 key-block pairs (DoubleRow)
NQC = S // 512       # 4 query chunks per batch
VW = HD + 1          # v columns per head incl. ones column
VP = 80              # padded per-head stride in v2 tiles
WSC = 32.0           # fp8 weight scale

Exp = mybir.ActivationFunctionType.Exp
DR = mybir.MatmulPerfMode.DoubleRow
MUL = mybir.AluOpType.mult
ADD = mybir.AluOpType.add
SUB = mybir.AluOpType.subtract

# Schraudolph exp->e4m3: bits = round(x * 8/ln2 + SCH_B)
SCH_A = 8.0 / np.log(2.0)
SCH_B = 55.63

# per-group exp engine schedule (16 key blocks): a=ACT, d=DVE
EXP_SCHED = "adaadaadaadaadaa"


def _build_nc(with_collective: bool = True):
    nc = bacc.Bacc("TRN2", target_bir_lowering=False, debug=False,
                   num_devices=NCORES)
    xt = nc.dram_tensor("xt", [D, SS], FP8, kind="ExternalInput").ap()
    xres = nc.dram_tensor("xres", [D, SS], FP8, kind="ExternalInput").ap()
    wqk = nc.dram_tensor("wqk", [D, 2 * FPC], FP8, kind="ExternalInput").ap()
    bqk = nc.dram_tensor("bqk", [128, 2], F32, kind="ExternalInput").ap()
    wv = nc.dram_tensor("wv", [D, FPC], FP8, kind="ExternalInput").ap()
    wvr = nc.dram_tensor("wvr", [D, FPC], FP8, kind="ExternalInput").ap()
    wout = nc.dram_tensor("wout", [D, D], BF16, kind="ExternalInput").ap()
    bob = nc.dram_tensor("bob", [128, D], F32, kind="ExternalInput").ap()
    ident = nc.dram_tensor("ident", [128, 128], BF16,
                           kind="ExternalInput").ap()
    onesr = nc.dram_tensor("onesr", [VW, HD], F32R,
                           kind="ExternalInput").ap()
    out = nc.dram_tensor("out", [SC, D], F32, kind="ExternalOutput").ap()

    with ExitStack() as ctx:
        tc = ctx.enter_context(tile.TileContext(nc))
        persist = ctx.enter_context(tc.tile_pool(name="persist", bufs=1))
        pexp = ctx.enter_context(tc.tile_pool(name="pexp", bufs=4))
        pwork = ctx.enter_context(tc.tile_pool(name="pwork", bufs=4))
        pbig = ctx.enter_context(tc.tile_pool(name="pbig", bufs=2, space="PSUM"))
        pso = ctx.enter_context(tc.tile_pool(name="pso", bufs=2, space="PSUM"))
        pps = ctx.enter_context(tc.tile_pool(name="pps", bufs=2, space="PSUM"))
        dram = ctx.enter_context(tc.tile_pool(name="dram", bufs=2, space="DRAM"))

        # ---------------- persistent SBUF ----------------
        # xt_sb[b]: [128, (i 8, s 2048)] fp8; xr_sb = fp8 residual
        xt_sb = [persist.tile([128, 8 * S], FP8, tag=f"xt{b}",
                              name=f"xt{b}") for b in range(B)]
        xt_r = [t.rearrange("p (i s) -> p i s", i=8) for t in xt_sb]
        xr_sb = [persist.tile([128, 8 * S], FP8, tag=f"xr{b}",
                              name=f"xr{b}") for b in range(B)]
        xr_r = [t.rearrange("p (i s) -> p i s", i=8) for t in xr_sb]
        wqk_sb = persist.tile([128, 8 * 2 * FPC], FP8, tag="wqk",
                              name="wqk_sb")
        wqk_r = wqk_sb.rearrange("p (i f) -> p i f", i=8)
        bqk_sb = persist.tile([128, 2], F32, tag="bqk", name="bqk")
        wv_sb = persist.tile([128, 8 * FPC], FP8, tag="wv", name="wv_sb")
        wv_r = wv_sb.rearrange("p (i f) -> p i f", i=8)
        wvr_sb = persist.tile([128, 8 * FPC], FP8, tag="wvr", name="wvr_sb")
        wvr_r = wvr_sb.rearrange("p (i f) -> p i f", i=8)
        wout_sb = persist.tile([128, 8 * D], BF16, tag="wout", name="wout_sb")
        wout_r = wout_sb.rearrange("p (i f) -> p i f", i=8)
        bob_sb = persist.tile([128, D], F32, tag="bob", name="bob")
        ident_sb = persist.tile([128, 128], BF16, tag="ident", name="ident")
        qk_sb = [[[persist.tile([128, 512], BF16, tag=f"qk{b}_{m}_{qn}",
                                name=f"qk{b}_{m}_{qn}") for qn in range(NQC)]
                  for m in range(2)]
                 for b in range(B)]
        # v2{hi,lo}[b][pi]: [128, 2*(2*VP)] fp8 - key-block pair pi; layout
        # [p, j(2), h(2), c(VP)], v at c 0:64; ones col 64: 1.0 in hi, 0 in lo
        v2hi = [[persist.tile([128, 2 * 2 * VP], FP8, tag=f"vh{b}_{pi}",
                              name=f"vh{b}_{pi}") for pi in range(NPAIR)]
                for b in range(B)]
        v2lo = [[persist.tile([128, 2 * 2 * VP], FP8, tag=f"vl{b}_{pi}",
                              name=f"vl{b}_{pi}") for pi in range(NPAIR)]
                for b in range(B)]
        v2hi_r = [[t.rearrange("p (j h c) -> p j h c", j=2, h=2)
                   for t in row] for row in v2hi]
        v2lo_r = [[t.rearrange("p (j h c) -> p j h c", j=2, h=2)
                   for t in row] for row in v2lo]
        att_sb = [[persist.tile([64, S], BF16, tag=f"att{b}_{h}",
                                name=f"att{b}_{h}") for h in range(HPC)]
                  for b in range(B)]
        ones32 = persist.tile([VW, HD], F32R, tag="ones32", name="ones32")
        nc.sync.dma_start(out=ones32, in_=onesr[:, :])
        ao_sb = [persist.tile([128, SC], BF16, tag=f"ao{i}", name=f"ao{i}")
                 for i in range(8)]
        part_sb = [persist.tile([128, 512], F32, tag=f"part{t}",
                                name=f"part{t}") for t in range(8)]
        for b in range(B):
            for pi in range(NPAIR):
                nc.gpsimd.memset(v2hi_r[b][pi][:, :, :, HD:VW], 1.0)
                nc.gpsimd.memset(v2lo_r[b][pi][:, :, :, HD:VW], 0.0)

        # ---------------- loads (ordered by first use) ----------------
        def load_x(dst_r, src, b, s0, s1, eng=None):
            # (p, i, s) -> src[i*128 + p, b*S + s]
            ap = bass.AP(tensor=src.tensor, offset=b * S + s0,
                         ap=[[SS, 128], [128 * SS, 8], [1, s1 - s0]])
            (eng or nc.sync).dma_start(out=dst_r[b][:, :, s0:s1], in_=ap)

        wqk_src = bass.AP(tensor=wqk.tensor, offset=0,
                          ap=[[2 * FPC, 128], [128 * 2 * FPC, 8], [1, 2 * FPC]])
        nc.sync.dma_start(out=wqk_r, in_=wqk_src)
        load_x(xt_r, xt, 0, 0, 512)
        load_x(xr_r, xres, 0, 0, 512)
        nc.sync.dma_start(out=bqk_sb, in_=bqk[:, :])
        load_x(xt_r, xt, 0, 512, 2048)
        load_x(xr_r, xres, 0, 512, 2048)
        wv_src = bass.AP(tensor=wv.tensor, offset=0,
                         ap=[[FPC, 128], [128 * FPC, 8], [1, FPC]])
        nc.sync.dma_start(out=wv_r, in_=wv_src)
        wvr_src = bass.AP(tensor=wvr.tensor, offset=0,
                          ap=[[FPC, 128], [128 * FPC, 8], [1, FPC]])
        nc.sync.dma_start(out=wvr_r, in_=wvr_src)
        nc.sync.dma_start(out=ident_sb, in_=ident[:, :])
        load_x(xt_r, xt, 1, 0, 2048)
        load_x(xr_r, xres, 1, 0, 2048)
        wout_src = bass.AP(tensor=wout.tensor, offset=0,
                           ap=[[D, 128], [128 * D, 8], [1, D]])
        nc.sync.dma_start(out=wout_r, in_=wout_src)
        nc.sync.dma_start(out=bob_sb, in_=bob[:, :])

        a2a_in = [dram.tile([8, HD, SC], BF16, tag=f"a2a_in{h}",
                            name=f"a2a_in{h}", bufs=1) for h in range(HPC)]
        a2a_out = [dram.tile([8, HD, SC], BF16, tag=f"a2a_out{h}",
                             name=f"a2a_out{h}", bufs=1) for h in range(HPC)]

        def emit_a2a(h):
            if with_collective:
                nc.gpsimd.collective_compute(
                    "AllToAll", mybir.AluOpType.bypass,
                    replica_groups=[list(range(8))],
                    ins=[a2a_in[h][:, :, :].opt()],
                    outs=[a2a_out[h][:, :, :].opt()])

        # ------------- projections + attention, interleaved -------------
        def emit_qk(b, m, qn):
            ps = pps.tile([128, 512], F32, tag="ps", name="ps_qk")
            for i in range(4):
                nc.tensor.matmul(
                    ps,
                    wqk_r[:, 2 * i:2 * i + 2, m * 128:(m + 1) * 128],
                    xt_r[b][:, 2 * i:2 * i + 2, qn * 512:(qn + 1) * 512],
                    start=(i == 0), stop=False, perf_mode=DR)
            for i in range(4):
                nc.tensor.matmul(
                    ps,
                    wqk_r[:, 2 * i:2 * i + 2, m * 128:(m + 1) * 128],
                    xr_r[b][:, 2 * i:2 * i + 2, qn * 512:(qn + 1) * 512],
                    start=False, stop=(i == 3), perf_mode=DR)
            nc.vector.tensor_scalar(
                qk_sb[b][m][qn], ps,
                (0.125 / WSC) if m == 0 else (1.0 / WSC),
                bqk_sb[:, m:m + 1], op0=MUL, op1=ADD)

        # v projection, operand-swapped: out [128 feat, 512 seq] in one
        # 12-matmul pass per (b, cq); transposed back per 128-seq block
        vfm_store = {}

        def emit_v_mm(b, cq):
            ps = pps.tile([128, 512], F32, tag="ps", name="ps_v")
            terms = [(wv_r, xt_r), (wvr_r, xt_r), (wv_r, xr_r)]
            n = 0
            for wsrc, xsrc in terms:
                for i in range(4):
                    nc.tensor.matmul(
                        ps,
                        wsrc[:, 2 * i:2 * i + 2, :],
                        xsrc[b][:, 2 * i:2 * i + 2,
                                cq * 512:(cq + 1) * 512],
                        start=(n == 0), stop=(n == 11), perf_mode=DR)
                    n += 1
            vfm = pwork.tile([128, 512], BF16, tag="vfm", name="vfm")
            if (b * 4 + cq) % 2:
                nc.scalar.copy(vfm, ps)
            else:
                nc.vector.tensor_copy(vfm, ps)
            vfm_store[(b, cq)] = vfm

        def emit_v_tr(b, cq):
            vfm = vfm_store.pop((b, cq))
            for s4 in range(4):
                sn = cq * 4 + s4
                pi, j = sn // 2, sn % 2
                pst = pps.tile([128, 128], BF16, tag="ps", name="pst")
                nc.tensor.transpose(
                    pst, vfm[:, s4 * 128:(s4 + 1) * 128], ident_sb)
                pst_r = pst.rearrange("p (h c) -> p h c", h=2)
                hi = v2hi_r[b][pi][:, j, :, 0:HD]
                nc.vector.tensor_copy(hi, pst_r)
                nc.vector.tensor_tensor(
                    v2lo_r[b][pi][:, j, :, 0:HD], pst_r, hi, op=SUB)

        def emit_attn(b, h, qh, fillers=()):
            # fillers: [(slot, thunk)]; popped at kb >= slot, so a filler's
            # products may only be consumed at kb >= slot (or later groups)
            fillers = sorted([e if isinstance(e, tuple) else (0, e)
                              for e in fillers], key=lambda e: e[0])
            pb = h * 64
            ps_o = [pso.tile([VW, 512], F32, tag="pso",
                             name=f"ps_o{q2}") for q2 in range(2)]

            def emit_av(pi, exv):
                for q2 in range(2):
                    nc.tensor.matmul(
                        ps_o[q2],
                        v2hi_r[b][pi][:, :, h, 0:VW],
                        exv[:, :, q2 * 512:(q2 + 1) * 512],
                        start=(pi == 0), stop=False, perf_mode=DR)
                    nc.tensor.matmul(
                        ps_o[q2],
                        v2lo_r[b][pi][:, :, h, 0:VW],
                        exv[:, :, q2 * 512:(q2 + 1) * 512],
                        start=False, stop=(pi == NPAIR - 1), perf_mode=DR)

            # attn@V for pair pi is emitted one pair late so PE never
            # hard-waits on the pair's last exp
            pend = None
            for pi in range(NPAIR):
                ex = pexp.tile([128, 2 * 1024], FP8, tag="expT",
                               name="expT")
                exv = ex.rearrange("p (j q) -> p j q", j=2)
                exu = ex.bitcast(U8).rearrange("p (j q) -> p j q", j=2)
                for j in range(2):
                    kb = 2 * pi + j
                    while fillers and fillers[0][0] <= kb:
                        fillers.pop(0)[1]()
                    ps_s = pbig.tile([128, 1024], F32, tag="scores",
                                     name="ps_s")
                    for q2 in range(2):
                        qc = qh * 2 + q2
                        nc.tensor.matmul(
                            ps_s[:, q2 * 512:(q2 + 1) * 512],
                            qk_sb[b][1][kb // 4][pb:pb + 64,
                                                 (kb % 4) * 128:
                                                 (kb % 4 + 1) * 128],
                            qk_sb[b][0][qc][pb:pb + 64, :],
                            start=True, stop=True)
                    if EXP_SCHED[kb] == "a":
                        nc.scalar.activation(exv[:, j, :], ps_s, Exp)
                    else:
                        nc.vector.tensor_scalar(
                            exu[:, j, :], ps_s, SCH_A, SCH_B,
                            op0=MUL, op1=ADD)
                    if j == 1 and pend is not None:
                        emit_av(*pend)
                pend = (pi, exv)
            emit_av(*pend)
            for _, f in fillers:
                f()
            # normalization head: recip + otmp now; bc+mult+ship deferred
            rec_s = pwork.tile([VW, 1024], F32R, tag="rec", name="rec_s")
            with nc.allow_low_precision(
                    reason="softmax denom recip rounded to f32r "
                           "for the PE broadcast"):
                for q2 in range(2):
                    nc.vector.reciprocal(
                        rec_s[HD:VW, q2 * 512:(q2 + 1) * 512],
                        ps_o[q2][HD:VW, :])
            otmps = []
            for q2 in range(2):
                otmp = pwork.tile([HD, 512], F32, tag="otmp",
                                  name="otmp")
                nc.scalar.copy(otmp, ps_o[q2][0:HD, :])
                otmps.append(otmp)

            def finish(q2):
                qc = qh * 2 + q2
                bc_ps = pps.tile([HD, 512], F32, tag="ps", name="bc_ps")
                nc.tensor.matmul(
                    bc_ps,
                    ones32[HD:VW, :],
                    rec_s[HD:VW, q2 * 512:(q2 + 1) * 512],
                    start=True, stop=True)
                nc.vector.tensor_tensor(
                    att_sb[b][h][:, qc * 512:(qc + 1) * 512],
                    otmps[q2], bc_ps[:, :], op=MUL)
                if qh == 1 and q2 == 1:
                    # (b,h) complete: ship the whole row in one DMA
                    nc.sync.dma_start(
                        out=a2a_in[h][b * 4:(b + 1) * 4, :, :].rearrange(
                            "j p s -> p j s"),
                        in_=att_sb[b][h].rearrange("p (j s) -> p j s", j=4))

            return [lambda: finish(0), lambda: finish(1)]

        srcb = a2a_out if with_collective else a2a_in

        def emit_unload(hi):
            for jj in range(4):
                nc.sync.dma_start(
                    out=ao_sb[4 * hi + jj],
                    in_=srcb[hi][2 * jj:2 * jj + 2, :, :].rearrange(
                        "j p s -> (j p) s"))

        # output projection tile t = sm*2 + en (bf16)
        def emit_out1(t):
            sm, en = t // 2, t % 2
            ps = pps.tile([128, 512], F32, tag="ps", name="ps_out1")
            for kk in range(4):
                nc.tensor.matmul(
                    ps, ao_sb[kk][:, sm * 128:(sm + 1) * 128],
                    wout_sb[:, kk * D + en * 512:kk * D + (en + 1) * 512],
                    start=(kk == 0), stop=(kk == 3))
            nc.vector.scalar_tensor_tensor(
                part_sb[t], ps, 1.0 / WSC,
                bob_sb[:, en * 512:(en + 1) * 512], op0=MUL, op1=ADD)

        def emit_out2(t):
            sm, en = t // 2, t % 2
            ps = pps.tile([128, 512], F32, tag="ps", name="ps_out2")
            for kk in range(4, 8):
                nc.tensor.matmul(
                    ps, ao_sb[kk][:, sm * 128:(sm + 1) * 128],
                    wout_sb[:, kk * D + en * 512:kk * D + (en + 1) * 512],
                    start=(kk == 4), stop=(kk == 7))
            osb = pwork.tile([128, 512], F32, tag="outsb", name="osb")
            nc.vector.scalar_tensor_tensor(
                osb, ps, 1.0 / WSC, part_sb[t], op0=MUL, op1=ADD)
            nc.sync.dma_start(
                out=out[sm * 128:(sm + 1) * 128, en * 512:(en + 1) * 512],
                in_=osb)

        def F(fn, *a):
            return lambda: fn(*a)

        # Filler safety rule: a filler popped at kb-slot i is emitted just
        # before slot i's scores, so anything it produces may only be
        # consumed at kb >= i (or by a later group).
        emit_qk(0, 1, 0)
        emit_qk(0, 1, 1)
        emit_qk(0, 0, 0)
        emit_qk(0, 0, 1)
        fin = emit_attn(0, 0, 0, fillers=(
            [(4 * cq, F(emit_v_mm, 0, cq)) for cq in range(4)]
            + [(4 * cq + 1, F(emit_v_tr, 0, cq)) for cq in range(4)]
            + [(6, F(emit_qk, 0, 1, 2)), (10, F(emit_qk, 0, 1, 3))]))
        emit_qk(0, 0, 2)
        emit_qk(0, 0, 3)
        fin = emit_attn(0, 1, 0, fillers=[(1, fin[0]), (3, fin[1])])
        fin = emit_attn(0, 0, 1, fillers=(
            [(1, fin[0]), (3, fin[1])]
            + [(i * 2, F(emit_qk, 1, 1, qn))
               for i, qn in enumerate(range(NQC))]
            + [(8, F(emit_qk, 1, 0, 0)), (10, F(emit_qk, 1, 0, 1))]))
        fin = emit_attn(0, 1, 1, fillers=(
            [(1, fin[0]), (3, fin[1])]
            + [(0, F(emit_v_mm, 1, 0)), (2, F(emit_v_tr, 1, 0)),
               (4, F(emit_v_mm, 1, 1)), (6, F(emit_v_tr, 1, 1))]
            + [(14, F(emit_qk, 1, 0, 2)), (15, F(emit_qk, 1, 0, 3))]))
        fin = emit_attn(1, 0, 0, fillers=(
            [(1, fin[0]), (3, fin[1])]
            + [(0, F(emit_v_mm, 1, 2)), (2, F(emit_v_tr, 1, 2)),
               (4, F(emit_v_mm, 1, 3)), (6, F(emit_v_tr, 1, 3))]))
        fin = emit_attn(1, 0, 1, fillers=[(1, fin[0]), (3, fin[1])])
        # (1,1,0) flushes the last even-head finishers -> a2a #0 can go
        fin = emit_attn(1, 1, 0, fillers=[(1, fin[0]), (3, fin[1])])
        emit_a2a(0)
        emit_unload(0)
        fin2 = emit_attn(1, 1, 1, fillers=(
            [(1, fin[0]), (3, fin[1])]
            + [(2 * t, F(emit_out1, t)) for t in range(8)]))
        for f in fin2:
            f()
        emit_a2a(1)
        emit_unload(1)
        for t in range(8):
            emit_out2(t)

    nc.compile()
    return nc


_NC_CACHE = {}


def _get_nc(with_collective: bool = True):
    key = bool(with_collective)
    if key not in _NC_CACHE:
        _NC_CACHE[key] = _build_nc(with_collective)
    return _NC_CACHE[key]


def make_in_maps(x, w_qkv, b_qkv, w_out, b_out):
    """Host-side sharding/prep. Returns per-core input dicts."""
    x = np.asarray(x, dtype=np.float32)
    w_qkv = np.asarray(w_qkv, dtype=np.float32)
    b_qkv = np.asarray(b_qkv, dtype=np.float32)
    w_out = np.asarray(w_out, dtype=np.float32)
    b_out = np.asarray(b_out, dtype=np.float32)

    wq = w_qkv[0:D].reshape(H, HD, D)
    wk = w_qkv[D:2 * D].reshape(H, HD, D)
    wv = w_qkv[2 * D:3 * D].reshape(H, HD, D)
    bq = b_qkv[0:D].reshape(H, HD)
    bk = b_qkv[D:2 * D].reshape(H, HD)
    scale = 1.0 / np.sqrt(HD)

    perm = np.concatenate(
        [np.arange(h * HD, (h + 1) * HD) for h in range(0, H, 2)]
        + [np.arange(h * HD, (h + 1) * HD) for h in range(1, H, 2)])
    wout_t = np.ascontiguousarray(w_out.T[perm]).astype(NPBF16)
    # v bias passes through attention unchanged (softmax rows sum to 1),
    # so its out-proj image folds into the output bias
    bv_full = b_qkv[2 * D:3 * D]
    bob_vec = b_out + w_out @ bv_full
    bob = np.ascontiguousarray(
        np.broadcast_to(bob_vec, (128, D))).astype(np.float32)

    # [d, 4096] stacked batch-major; fp8 hi + fp8 residual
    xt_f32 = np.ascontiguousarray(
        np.concatenate([x[0].T, x[1].T], axis=1))
    xt_all = xt_f32.astype(NPFP8)
    xres_all = (xt_f32 - xt_all.astype(np.float32)).astype(NPFP8)

    in_maps = []
    for c in range(NCORES):
        hs = slice(c * HPC, (c + 1) * HPC)
        wq_c = (wq[hs].reshape(FPC, D) * WSC).T
        wk_c = (wk[hs].reshape(FPC, D) * WSC).T
        wqk_c = np.concatenate([wq_c, wk_c], axis=1).astype(NPFP8)
        bqk_c = np.concatenate([bq[hs].reshape(FPC) * scale,
                                bk[hs].reshape(FPC)])
        bqk_c = np.ascontiguousarray(
            bqk_c.reshape(2, 128).T).astype(np.float32)
        wv_f = (wv[hs].reshape(FPC, D) * WSC).T
        wv_c = wv_f.astype(NPFP8)
        wvr_c = (wv_f - wv_c.astype(np.float32)).astype(NPFP8)
        in_maps.append({
            "onesr": np.ones((VW, HD), dtype=np.float32),
            "ident": np.eye(128, dtype=np.float32).astype(NPBF16),
            "xt": xt_all,
            "xres": xres_all,
            "wqk": np.ascontiguousarray(wqk_c),
            "bqk": bqk_c,
            "wv": np.ascontiguousarray(wv_c),
            "wvr": np.ascontiguousarray(wvr_c),
            "wout": wout_t,
            "bob": bob,
        })
    return in_maps


def assemble_output(results):
    out = np.empty((B, S, D), dtype=np.float32)
    for c in range(NCORES):
        b, sg = c // 4, c % 4
        out[b, sg * SC:(sg + 1) * SC, :] = results[c]["out"]
    return out


def kernel(x, mask, w_qkv, b_qkv, w_out, b_out):
    nc = _get_nc(True)
    in_maps = make_in_maps(x, w_qkv, b_qkv, w_out, b_out)
    res = run_bass_kernel_spmd(nc, in_maps, core_ids=list(range(NCORES)))
    return assemble_output(res.results)


# revision 46
# speedup vs baseline: 1.0358x; 1.0147x over previous
"""Multi-head attention on 8 Trainium2 NeuronCores (Bass/Tile), fp8 edition.

Problem: x[2,2048,1024] -> qkv proj (16 heads, hd=64) -> softmax(QK^T/8)V
-> out proj.  mask is all-ones (per spec) and is ignored.

Sharding: core c owns heads {2c, 2c+1} for BOTH batches (tensor-parallel
QKV + attention).  An 8-core AllToAll converts the head-sharded attention
output into a sequence-sharded full-feature activation; core c ends up
with global row chunk c (batch c//4, rows (c%4)*512..) and computes the
output projection full-width with no all-reduce.

Precision/speed strategy (cost model: fp8e4 DoubleRow matmul = 0.5
cycles/row, i.e. 4x bf16 per FLOP at K-pair packing):
  - qk projection: fp8 DoubleRow, 2-term (x8@w8 + xres8@w8) to keep
    q/k noise small; score-side noise averages through softmax.
  - v projection: fp8 DoubleRow, 3-term (x8@wv8 + x8@wvres + xres8@wv8).
  - scores: bf16 (hd=64 contraction can't DoubleRow without expensive
    partition-split copies).
  - exp(scores)->fp8 probabilities on TWO engines: ACT (native Exp, fp8
    out) and DVE (Schraudolph: u8 = round(x*8/ln2 + 55.63) IS the e4m3
    encoding of ~e^x).  GPSIMD can't read PSUM, so it gets no exp work.
  - attn@V: fp8 DoubleRow over key-block pairs, with V stored hi+lo
    (v2hi = fp8(32v), v2lo = fp8(32v - v2hi)) accumulating into the SAME
    PSUM group -> v quantization error cancels.  A ones column in v2hi
    (zeros in v2lo) accumulates the softmax denominator in PSUM row 64.
  - att = 32*num/den in bf16; out projection in plain bf16.
  - v bias and out bias fold into bob = b_out + w_out@bv host-side.

Schedule: normalization finishers (bc matmul + mult + ship chunk) of
each attention group are DEFERRED into the next group's filler slots so
PE never stalls on the recip; output DMAs are split into 128KB chunks
to shrink the serial tail.

PSUM: scores [128,1024] x2 (4 banks) + pso [65,512] x2 (2) + pps
[128,512] x2 (2, also hosts bc broadcasts) = 8 banks.
"""

import numpy as np
import ml_dtypes
from contextlib import ExitStack

import concourse.bass as bass
import concourse.mybir as mybir
import concourse.tile as tile
from concourse import bacc
from concourse.bass_utils import run_bass_kernel_spmd

BF16 = mybir.dt.bfloat16
F32 = mybir.dt.float32
F32R = mybir.dt.float32r
FP8 = mybir.dt.float8e4
U8 = mybir.dt.uint8
NPBF16 = ml_dtypes.bfloat16
NPFP8 = ml_dtypes.float8_e4m3

D, H, HD, B, S = 1024, 16, 64, 2, 2048
NCORES = 8
HPC = 2              # heads per core
FPC = HPC * HD       # 128 features per core
SS = B * S           # 4096 stacked sequence (batch-major)
SC = 512             # output rows per core (post all-to-all)
NKB = S // 128       # 16 key blocks per batch
NPAIR = NKB // 2     # 8 key-block pairs (DoubleRow)
NQC = S // 512       # 4 query chunks per batch
VW = HD + 1          # v columns per head incl. ones column
VP = 80              # padded per-head stride in v2 tiles
WSC = 32.0           # fp8 weight scale

Exp = mybir.ActivationFunctionType.Exp
DR = mybir.MatmulPerfMode.DoubleRow
MUL = mybir.AluOpType.mult
ADD = mybir.AluOpType.add
SUB = mybir.AluOpType.subtract

# Schraudolph exp->e4m3: bits = round(x * 8/ln2 + SCH_B)
SCH_A = 8.0 / np.log(2.0)
SCH_B = 55.63

# per-group exp engine schedule (16 key blocks): a=ACT, d=DVE
EXP_SCHED = "adaadaadaadaadaa"


def _build_nc(with_collective: bool = True):
    nc = bacc.Bacc("TRN2", target_bir_lowering=False, debug=False,
                   num_devices=NCORES)
    xt = nc.dram_tensor("xt", [D, SS], FP8, kind="ExternalInput").ap()
    xres = nc.dram_tensor("xres", [D, SS], FP8, kind="ExternalInput").ap()
    wqk = nc.dram_tensor("wqk", [D, 2 * FPC], FP8, kind="ExternalInput").ap()
    bqk = nc.dram_tensor("bqk", [128, 2], F32, kind="ExternalInput").ap()
    wv = nc.dram_tensor("wv", [D, FPC], FP8, kind="ExternalInput").ap()
    wvr = nc.dram_tensor("wvr", [D, FPC], FP8, kind="ExternalInput").ap()
    wout = nc.dram_tensor("wout", [D, D], BF16, kind="ExternalInput").ap()
    bob = nc.dram_tensor("bob", [128, D], F32, kind="ExternalInput").ap()
    ident = nc.dram_tensor("ident", [128, 128], BF16,
                           kind="ExternalInput").ap()
    onesr = nc.dram_tensor("onesr", [VW, HD], F32R,
                           kind="ExternalInput").ap()
    out = nc.dram_tensor("out", [SC, D], F32, kind="ExternalOutput").ap()

    with ExitStack() as ctx:
        tc = ctx.enter_context(tile.TileContext(nc))
        persist = ctx.enter_context(tc.tile_pool(name="persist", bufs=1))
        pexp = ctx.enter_context(tc.tile_pool(name="pexp", bufs=4))
        pwork = ctx.enter_context(tc.tile_pool(name="pwork", bufs=4))
        pbig = ctx.enter_context(tc.tile_pool(name="pbig", bufs=2, space="PSUM"))
        pso = ctx.enter_context(tc.tile_pool(name="pso", bufs=2, space="PSUM"))
        pps = ctx.enter_context(tc.tile_pool(name="pps", bufs=2, space="PSUM"))
        dram = ctx.enter_context(tc.tile_pool(name="dram", bufs=2, space="DRAM"))

        # ---------------- persistent SBUF ----------------
        # xt_sb[b]: [128, (i 8, s 2048)] fp8; xr_sb = fp8 residual
        xt_sb = [persist.tile([128, 8 * S], FP8, tag=f"xt{b}",
                              name=f"xt{b}") for b in range(B)]
        xt_r = [t.rearrange("p (i s) -> p i s", i=8) for t in xt_sb]
        xr_sb = [persist.tile([128, 8 * S], FP8, tag=f"xr{b}",
                              name=f"xr{b}") for b in range(B)]
        xr_r = [t.rearrange("p (i s) -> p i s", i=8) for t in xr_sb]
        wqk_sb = persist.tile([128, 8 * 2 * FPC], FP8, tag="wqk",
                              name="wqk_sb")
        wqk_r = wqk_sb.rearrange("p (i f) -> p i f", i=8)
        bqk_sb = persist.tile([128, 2], F32, tag="bqk", name="bqk")
        wv_sb = persist.tile([128, 8 * FPC], FP8, tag="wv", name="wv_sb")
        wv_r = wv_sb.rearrange("p (i f) -> p i f", i=8)
        wvr_sb = persist.tile([128, 8 * FPC], FP8, tag="wvr", name="wvr_sb")
        wvr_r = wvr_sb.rearrange("p (i f) -> p i f", i=8)
        wout_sb = persist.tile([128, 8 * D], BF16, tag="wout", name="wout_sb")
        wout_r = wout_sb.rearrange("p (i f) -> p i f", i=8)
        bob_sb = persist.tile([128, D], F32, tag="bob", name="bob")
        ident_sb = persist.tile([128, 128], BF16, tag="ident", name="ident")
        qk_sb = [[[persist.tile([128, 512], BF16, tag=f"qk{b}_{m}_{qn}",
                                name=f"qk{b}_{m}_{qn}") for qn in range(NQC)]
                  for m in range(2)]
                 for b in range(B)]
        # v2{hi,lo}[b][pi]: [128, 2*(2*VP)] fp8 - key-block pair pi; layout
        # [p, j(2), h(2), c(VP)], v at c 0:64; ones col 64: 1.0 in hi, 0 in lo
        v2hi = [[persist.tile([128, 2 * 2 * VP], FP8, tag=f"vh{b}_{pi}",
                              name=f"vh{b}_{pi}") for pi in range(NPAIR)]
                for b in range(B)]
        v2lo = [[persist.tile([128, 2 * 2 * VP], FP8, tag=f"vl{b}_{pi}",
                              name=f"vl{b}_{pi}") for pi in range(NPAIR)]
                for b in range(B)]
        v2hi_r = [[t.rearrange("p (j h c) -> p j h c", j=2, h=2)
                   for t in row] for row in v2hi]
        v2lo_r = [[t.rearrange("p (j h c) -> p j h c", j=2, h=2)
                   for t in row] for row in v2lo]
        att_sb = [[persist.tile([64, S], BF16, tag=f"att{b}_{h}",
                                name=f"att{b}_{h}") for h in range(HPC)]
                  for b in range(B)]
        ones32 = persist.tile([VW, HD], F32R, tag="ones32", name="ones32")
        nc.sync.dma_start(out=ones32, in_=onesr[:, :])
        ao_sb = [persist.tile([128, SC], BF16, tag=f"ao{i}", name=f"ao{i}")
                 for i in range(8)]
        part_sb = [persist.tile([128, 512], F32, tag=f"part{t}",
                                name=f"part{t}") for t in range(8)]
        for b in range(B):
            for pi in range(NPAIR):
                nc.gpsimd.memset(v2hi_r[b][pi][:, :, :, HD:VW], 1.0)
                nc.gpsimd.memset(v2lo_r[b][pi][:, :, :, HD:VW], 0.0)

        # ---------------- loads (ordered by first use) ----------------
        def load_x(dst_r, src, b, s0, s1, eng=None):
            # (p, i, s) -> src[i*128 + p, b*S + s]
            ap = bass.AP(tensor=src.tensor, offset=b * S + s0,
                         ap=[[SS, 128], [128 * SS, 8], [1, s1 - s0]])
            (eng or nc.sync).dma_start(out=dst_r[b][:, :, s0:s1], in_=ap)

        wqk_src = bass.AP(tensor=wqk.tensor, offset=0,
                          ap=[[2 * FPC, 128], [128 * 2 * FPC, 8], [1, 2 * FPC]])
        nc.sync.dma_start(out=wqk_r, in_=wqk_src)
        load_x(xt_r, xt, 0, 0, 1024)
        load_x(xr_r, xres, 0, 0, 1024)
        nc.sync.dma_start(out=bqk_sb, in_=bqk[:, :])
        load_x(xt_r, xt, 0, 1024, 2048)
        load_x(xr_r, xres, 0, 1024, 2048)
        wv_src = bass.AP(tensor=wv.tensor, offset=0,
                         ap=[[FPC, 128], [128 * FPC, 8], [1, FPC]])
        nc.sync.dma_start(out=wv_r, in_=wv_src)
        wvr_src = bass.AP(tensor=wvr.tensor, offset=0,
                          ap=[[FPC, 128], [128 * FPC, 8], [1, FPC]])
        nc.sync.dma_start(out=wvr_r, in_=wvr_src)
        nc.sync.dma_start(out=ident_sb, in_=ident[:, :])
        load_x(xt_r, xt, 1, 0, 2048)
        load_x(xr_r, xres, 1, 0, 2048)
        wout_src = bass.AP(tensor=wout.tensor, offset=0,
                           ap=[[D, 128], [128 * D, 8], [1, D]])
        nc.sync.dma_start(out=wout_r, in_=wout_src)
        nc.sync.dma_start(out=bob_sb, in_=bob[:, :])

        a2a_in = [dram.tile([8, HD, SC], BF16, tag=f"a2a_in{h}",
                            name=f"a2a_in{h}", bufs=1) for h in range(HPC)]
        a2a_out = [dram.tile([8, HD, SC], BF16, tag=f"a2a_out{h}",
                             name=f"a2a_out{h}", bufs=1) for h in range(HPC)]

        def emit_a2a(h):
            if with_collective:
                nc.gpsimd.collective_compute(
                    "AllToAll", mybir.AluOpType.bypass,
                    replica_groups=[list(range(8))],
                    ins=[a2a_in[h][:, :, :].opt()],
                    outs=[a2a_out[h][:, :, :].opt()])

        # ------------- projections + attention, interleaved -------------
        def emit_qk(b, m, qn):
            ps = pps.tile([128, 512], F32, tag="ps", name="ps_qk")
            for i in range(4):
                nc.tensor.matmul(
                    ps,
                    wqk_r[:, 2 * i:2 * i + 2, m * 128:(m + 1) * 128],
                    xt_r[b][:, 2 * i:2 * i + 2, qn * 512:(qn + 1) * 512],
                    start=(i == 0), stop=False, perf_mode=DR)
            for i in range(4):
                nc.tensor.matmul(
                    ps,
                    wqk_r[:, 2 * i:2 * i + 2, m * 128:(m + 1) * 128],
                    xr_r[b][:, 2 * i:2 * i + 2, qn * 512:(qn + 1) * 512],
                    start=False, stop=(i == 3), perf_mode=DR)
            nc.vector.tensor_scalar(
                qk_sb[b][m][qn], ps,
                (0.125 / WSC) if m == 0 else (1.0 / WSC),
                bqk_sb[:, m:m + 1], op0=MUL, op1=ADD)

        # v projection, operand-swapped: out [128 feat, 512 seq] in one
        # 12-matmul pass per (b, cq); transposed back per 128-seq block
        vfm_store = {}

        def emit_v_mm(b, cq):
            ps = pps.tile([128, 512], F32, tag="ps", name="ps_v")
            terms = [(wv_r, xt_r), (wvr_r, xt_r), (wv_r, xr_r)]
            n = 0
            for wsrc, xsrc in terms:
                for i in range(4):
                    nc.tensor.matmul(
                        ps,
                        wsrc[:, 2 * i:2 * i + 2, :],
                        xsrc[b][:, 2 * i:2 * i + 2,
                                cq * 512:(cq + 1) * 512],
                        start=(n == 0), stop=(n == 11), perf_mode=DR)
                    n += 1
            vfm = pwork.tile([128, 512], BF16, tag="vfm", name="vfm")
            if (b * 4 + cq) % 2:
                nc.scalar.copy(vfm, ps)
            else:
                nc.vector.tensor_copy(vfm, ps)
            vfm_store[(b, cq)] = vfm

        def emit_v_tr(b, cq):
            vfm = vfm_store.pop((b, cq))
            for s4 in range(4):
                sn = cq * 4 + s4
                pi, j = sn // 2, sn % 2
                pst = pps.tile([128, 128], BF16, tag="ps", name="pst")
                nc.tensor.transpose(
                    pst, vfm[:, s4 * 128:(s4 + 1) * 128], ident_sb)
                pst_r = pst.rearrange("p (h c) -> p h c", h=2)
                hi = v2hi_r[b][pi][:, j, :, 0:HD]
                nc.vector.tensor_copy(hi, pst_r)
                nc.vector.tensor_tensor(
                    v2lo_r[b][pi][:, j, :, 0:HD], pst_r, hi, op=SUB)

        def emit_attn(b, h, qh, fillers=()):
            # fillers: [(slot, thunk)]; popped at kb >= slot, so a filler's
            # products may only be consumed at kb >= slot (or later groups)
            fillers = sorted([e if isinstance(e, tuple) else (0, e)
                              for e in fillers], key=lambda e: e[0])
            pb = h * 64
            ps_o = [pso.tile([VW, 512], F32, tag="pso",
                             name=f"ps_o{q2}") for q2 in range(2)]

            def emit_av(pi, exv):
                for q2 in range(2):
                    nc.tensor.matmul(
                        ps_o[q2],
                        v2hi_r[b][pi][:, :, h, 0:VW],
                        exv[:, :, q2 * 512:(q2 + 1) * 512],
                        start=(pi == 0), stop=False, perf_mode=DR)
                    nc.tensor.matmul(
                        ps_o[q2],
                        v2lo_r[b][pi][:, :, h, 0:VW],
                        exv[:, :, q2 * 512:(q2 + 1) * 512],
                        start=False, stop=(pi == NPAIR - 1), perf_mode=DR)

            # attn@V for pair pi is emitted one pair late so PE never
            # hard-waits on the pair's last exp
            pend = None
            for pi in range(NPAIR):
                ex = pexp.tile([128, 2 * 1024], FP8, tag="expT",
                               name="expT")
                exv = ex.rearrange("p (j q) -> p j q", j=2)
                exu = ex.bitcast(U8).rearrange("p (j q) -> p j q", j=2)
                for j in range(2):
                    kb = 2 * pi + j
                    while fillers and fillers[0][0] <= kb:
                        fillers.pop(0)[1]()
                    ps_s = pbig.tile([128, 1024], F32, tag="scores",
                                     name="ps_s")
                    for q2 in range(2):
                        qc = qh * 2 + q2
                        nc.tensor.matmul(
                            ps_s[:, q2 * 512:(q2 + 1) * 512],
                            qk_sb[b][1][kb // 4][pb:pb + 64,
                                                 (kb % 4) * 128:
                                                 (kb % 4 + 1) * 128],
                            qk_sb[b][0][qc][pb:pb + 64, :],
                            start=True, stop=True)
                    if EXP_SCHED[kb] == "a":
                        nc.scalar.activation(exv[:, j, :], ps_s, Exp)
                    else:
                        nc.vector.tensor_scalar(
                            exu[:, j, :], ps_s, SCH_A, SCH_B,
                            op0=MUL, op1=ADD)
                    if j == 1 and pend is not None:
                        emit_av(*pend)
                pend = (pi, exv)
            emit_av(*pend)
            for _, f in fillers:
                f()
            # normalization head: recip + otmp now; bc+mult+ship deferred
            rec_s = pwork.tile([VW, 1024], F32R, tag="rec", name="rec_s")
            with nc.allow_low_precision(
                    reason="softmax denom recip rounded to f32r "
                           "for the PE broadcast"):
                for q2 in range(2):
                    nc.vector.reciprocal(
                        rec_s[HD:VW, q2 * 512:(q2 + 1) * 512],
                        ps_o[q2][HD:VW, :])
            otmps = []
            for q2 in range(2):
                otmp = pwork.tile([HD, 512], F32, tag="otmp",
                                  name="otmp")
                nc.scalar.copy(otmp, ps_o[q2][0:HD, :])
                otmps.append(otmp)

            def finish(q2):
                qc = qh * 2 + q2
                bc_ps = pps.tile([HD, 512], F32, tag="ps", name="bc_ps")
                nc.tensor.matmul(
                    bc_ps,
                    ones32[HD:VW, :],
                    rec_s[HD:VW, q2 * 512:(q2 + 1) * 512],
                    start=True, stop=True)
                nc.vector.tensor_tensor(
                    att_sb[b][h][:, qc * 512:(qc + 1) * 512],
                    otmps[q2], bc_ps[:, :], op=MUL)
                if qh == 1 and q2 == 1:
                    # (b,h) complete: ship the whole row in one DMA
                    nc.sync.dma_start(
                        out=a2a_in[h][b * 4:(b + 1) * 4, :, :].rearrange(
                            "j p s -> p j s"),
                        in_=att_sb[b][h].rearrange("p (j s) -> p j s", j=4))

            return [lambda: finish(0), lambda: finish(1)]

        srcb = a2a_out if with_collective else a2a_in

        def emit_unload(hi):
            for jj in range(4):
                nc.sync.dma_start(
                    out=ao_sb[4 * hi + jj],
                    in_=srcb[hi][2 * jj:2 * jj + 2, :, :].rearrange(
                        "j p s -> (j p) s"))

        # output projection tile t = sm*2 + en (bf16)
        def emit_out1(t):
            sm, en = t // 2, t % 2
            ps = pps.tile([128, 512], F32, tag="ps", name="ps_out1")
            for kk in range(4):
                nc.tensor.matmul(
                    ps, ao_sb[kk][:, sm * 128:(sm + 1) * 128],
                    wout_sb[:, kk * D + en * 512:kk * D + (en + 1) * 512],
                    start=(kk == 0), stop=(kk == 3))
            nc.vector.scalar_tensor_tensor(
                part_sb[t], ps, 1.0 / WSC,
                bob_sb[:, en * 512:(en + 1) * 512], op0=MUL, op1=ADD)

        def emit_out2(t):
            sm, en = t // 2, t % 2
            ps = pps.tile([128, 512], F32, tag="ps", name="ps_out2")
            for kk in range(4, 8):
                nc.tensor.matmul(
                    ps, ao_sb[kk][:, sm * 128:(sm + 1) * 128],
                    wout_sb[:, kk * D + en * 512:kk * D + (en + 1) * 512],
                    start=(kk == 4), stop=(kk == 7))
            osb = pwork.tile([128, 512], F32, tag="outsb", name="osb")
            nc.vector.scalar_tensor_tensor(
                osb, ps, 1.0 / WSC, part_sb[t], op0=MUL, op1=ADD)
            nc.sync.dma_start(
                out=out[sm * 128:(sm + 1) * 128, en * 512:(en + 1) * 512],
                in_=osb)

        def F(fn, *a):
            return lambda: fn(*a)

        # Filler safety rule: a filler popped at kb-slot i is emitted just
        # before slot i's scores, so anything it produces may only be
        # consumed at kb >= i (or by a later group).
        emit_qk(0, 1, 0)
        emit_qk(0, 1, 1)
        emit_qk(0, 0, 0)
        emit_qk(0, 0, 1)
        fin = emit_attn(0, 0, 0, fillers=(
            [(4 * cq, F(emit_v_mm, 0, cq)) for cq in range(4)]
            + [(4 * cq + 1, F(emit_v_tr, 0, cq)) for cq in range(4)]
            + [(6, F(emit_qk, 0, 1, 2)), (10, F(emit_qk, 0, 1, 3))]))
        emit_qk(0, 0, 2)
        emit_qk(0, 0, 3)
        fin = emit_attn(0, 1, 0, fillers=[(1, fin[0]), (3, fin[1])])
        fin = emit_attn(0, 0, 1, fillers=(
            [(1, fin[0]), (3, fin[1])]
            + [(i * 2, F(emit_qk, 1, 1, qn))
               for i, qn in enumerate(range(NQC))]
            + [(8, F(emit_qk, 1, 0, 0)), (10, F(emit_qk, 1, 0, 1))]))
        fin = emit_attn(0, 1, 1, fillers=(
            [(1, fin[0]), (3, fin[1])]
            + [(0, F(emit_v_mm, 1, 0)), (2, F(emit_v_tr, 1, 0)),
               (4, F(emit_v_mm, 1, 1)), (6, F(emit_v_tr, 1, 1))]
            + [(14, F(emit_qk, 1, 0, 2)), (15, F(emit_qk, 1, 0, 3))]))
        fin = emit_attn(1, 0, 0, fillers=(
            [(1, fin[0]), (3, fin[1])]
            + [(0, F(emit_v_mm, 1, 2)), (2, F(emit_v_tr, 1, 2)),
               (4, F(emit_v_mm, 1, 3)), (6, F(emit_v_tr, 1, 3))]))
        fin = emit_attn(1, 0, 1, fillers=[(1, fin[0]), (3, fin[1])])
        # (1,1,0) flushes the last even-head finishers -> a2a #0 can go
        fin = emit_attn(1, 1, 0, fillers=[(1, fin[0]), (3, fin[1])])
        emit_a2a(0)
        emit_unload(0)
        fin2 = emit_attn(1, 1, 1, fillers=(
            [(1, fin[0]), (3, fin[1])]
            + [(2 * t, F(emit_out1, t)) for t in range(8)]))
        for f in fin2:
            f()
        emit_a2a(1)
        emit_unload(1)
        for t in range(8):
            emit_out2(t)

    nc.compile()
    return nc


_NC_CACHE = {}


def _get_nc(with_collective: bool = True):
    key = bool(with_collective)
    if key not in _NC_CACHE:
        _NC_CACHE[key] = _build_nc(with_collective)
    return _NC_CACHE[key]


def make_in_maps(x, w_qkv, b_qkv, w_out, b_out):
    """Host-side sharding/prep. Returns per-core input dicts."""
    x = np.asarray(x, dtype=np.float32)
    w_qkv = np.asarray(w_qkv, dtype=np.float32)
    b_qkv = np.asarray(b_qkv, dtype=np.float32)
    w_out = np.asarray(w_out, dtype=np.float32)
    b_out = np.asarray(b_out, dtype=np.float32)

    wq = w_qkv[0:D].reshape(H, HD, D)
    wk = w_qkv[D:2 * D].reshape(H, HD, D)
    wv = w_qkv[2 * D:3 * D].reshape(H, HD, D)
    bq = b_qkv[0:D].reshape(H, HD)
    bk = b_qkv[D:2 * D].reshape(H, HD)
    scale = 1.0 / np.sqrt(HD)

    perm = np.concatenate(
        [np.arange(h * HD, (h + 1) * HD) for h in range(0, H, 2)]
        + [np.arange(h * HD, (h + 1) * HD) for h in range(1, H, 2)])
    wout_t = np.ascontiguousarray(w_out.T[perm]).astype(NPBF16)
    # v bias passes through attention unchanged (softmax rows sum to 1),
    # so its out-proj image folds into the output bias
    bv_full = b_qkv[2 * D:3 * D]
    bob_vec = b_out + w_out @ bv_full
    bob = np.ascontiguousarray(
        np.broadcast_to(bob_vec, (128, D))).astype(np.float32)

    # [d, 4096] stacked batch-major; fp8 hi + fp8 residual
    xt_f32 = np.ascontiguousarray(
        np.concatenate([x[0].T, x[1].T], axis=1))
    xt_all = xt_f32.astype(NPFP8)
    xres_all = (xt_f32 - xt_all.astype(np.float32)).astype(NPFP8)

    in_maps = []
    for c in range(NCORES):
        hs = slice(c * HPC, (c + 1) * HPC)
        wq_c = (wq[hs].reshape(FPC, D) * WSC).T
        wk_c = (wk[hs].reshape(FPC, D) * WSC).T
        wqk_c = np.concatenate([wq_c, wk_c], axis=1).astype(NPFP8)
        bqk_c = np.concatenate([bq[hs].reshape(FPC) * scale,
                                bk[hs].reshape(FPC)])
        bqk_c = np.ascontiguousarray(
            bqk_c.reshape(2, 128).T).astype(np.float32)
        wv_f = (wv[hs].reshape(FPC, D) * WSC).T
        wv_c = wv_f.astype(NPFP8)
        wvr_c = (wv_f - wv_c.astype(np.float32)).astype(NPFP8)
        in_maps.append({
            "onesr": np.ones((VW, HD), dtype=np.float32),
            "ident": np.eye(128, dtype=np.float32).astype(NPBF16),
            "xt": xt_all,
            "xres": xres_all,
            "wqk": np.ascontiguousarray(wqk_c),
            "bqk": bqk_c,
            "wv": np.ascontiguousarray(wv_c),
            "wvr": np.ascontiguousarray(wvr_c),
            "wout": wout_t,
            "bob": bob,
        })
    return in_maps


def assemble_output(results):
    out = np.empty((B, S, D), dtype=np.float32)
    for c in range(NCORES):
        b, sg = c // 4, c % 4
        out[b, sg * SC:(sg + 1) * SC, :] = results[c]["out"]
    return out


def kernel(x, mask, w_qkv, b_qkv, w_out, b_out):
    nc = _get_nc(True)
    in_maps = make_in_maps(x, w_qkv, b_qkv, w_out, b_out)
    res = run_bass_kernel_spmd(nc, in_maps, core_ids=list(range(NCORES)))
    return assemble_output(res.results)


# revision 47
# speedup vs baseline: 1.0388x; 1.0028x over previous
"""Multi-head attention on 8 Trainium2 NeuronCores (Bass/Tile), fp8 edition.

Problem: x[2,2048,1024] -> qkv proj (16 heads, hd=64) -> softmax(QK^T/8)V
-> out proj.  mask is all-ones (per spec) and is ignored.

Sharding: core c owns heads {2c, 2c+1} for BOTH batches (tensor-parallel
QKV + attention).  An 8-core AllToAll converts the head-sharded attention
output into a sequence-sharded full-feature activation; core c ends up
with global row chunk c (batch c//4, rows (c%4)*512..) and computes the
output projection full-width with no all-reduce.

Precision/speed strategy (cost model: fp8e4 DoubleRow matmul = 0.5
cycles/row, i.e. 4x bf16 per FLOP at K-pair packing):
  - qk projection: fp8 DoubleRow, 2-term (x8@w8 + xres8@w8) to keep
    q/k noise small; score-side noise averages through softmax.
  - v projection: fp8 DoubleRow, 3-term (x8@wv8 + x8@wvres + xres8@wv8).
  - scores: bf16 (hd=64 contraction can't DoubleRow without expensive
    partition-split copies).
  - exp(scores)->fp8 probabilities on TWO engines: ACT (native Exp, fp8
    out) and DVE (Schraudolph: u8 = round(x*8/ln2 + 55.63) IS the e4m3
    encoding of ~e^x).  GPSIMD can't read PSUM, so it gets no exp work.
  - attn@V: fp8 DoubleRow over key-block pairs, with V stored hi+lo
    (v2hi = fp8(32v), v2lo = fp8(32v - v2hi)) accumulating into the SAME
    PSUM group -> v quantization error cancels.  A ones column in v2hi
    (zeros in v2lo) accumulates the softmax denominator in PSUM row 64.
  - att = 32*num/den in bf16; out projection in plain bf16.
  - v bias and out bias fold into bob = b_out + w_out@bv host-side.

Schedule: normalization finishers (bc matmul + mult + ship chunk) of
each attention group are DEFERRED into the next group's filler slots so
PE never stalls on the recip; output DMAs are split into 128KB chunks
to shrink the serial tail.

PSUM: scores [128,1024] x2 (4 banks) + pso [65,512] x2 (2) + pps
[128,512] x2 (2, also hosts bc broadcasts) = 8 banks.
"""

import numpy as np
import ml_dtypes
from contextlib import ExitStack

import concourse.bass as bass
import concourse.mybir as mybir
import concourse.tile as tile
from concourse import bacc
from concourse.bass_utils import run_bass_kernel_spmd

BF16 = mybir.dt.bfloat16
F32 = mybir.dt.float32
F32R = mybir.dt.float32r
FP8 = mybir.dt.float8e4
U8 = mybir.dt.uint8
NPBF16 = ml_dtypes.bfloat16
NPFP8 = ml_dtypes.float8_e4m3

D, H, HD, B, S = 1024, 16, 64, 2, 2048
NCORES = 8
HPC = 2              # heads per core
FPC = HPC * HD       # 128 features per core
SS = B * S           # 4096 stacked sequence (batch-major)
SC = 512             # output rows per core (post all-to-all)
NKB = S // 128       # 16 key blocks per batch
NPAIR = NKB // 2     # 8 key-block pairs (DoubleRow)
NQC = S // 512       # 4 query chunks per batch
VW = HD + 1          # v columns per head incl. ones column
VP = 80              # padded per-head stride in v2 tiles
WSC = 32.0           # fp8 weight scale

Exp = mybir.ActivationFunctionType.Exp
DR = mybir.MatmulPerfMode.DoubleRow
MUL = mybir.AluOpType.mult
ADD = mybir.AluOpType.add
SUB = mybir.AluOpType.subtract

# Schraudolph exp->e4m3: bits = round(x * 8/ln2 + SCH_B)
SCH_A = 8.0 / np.log(2.0)
SCH_B = 55.63

# per-group exp engine schedule (16 key blocks): a=ACT, d=DVE
EXP_SCHED = "adaadaadaadaadaa"


def _build_nc(with_collective: bool = True):
    nc = bacc.Bacc("TRN2", target_bir_lowering=False, debug=False,
                   num_devices=NCORES)
    xt = nc.dram_tensor("xt", [D, SS], FP8, kind="ExternalInput").ap()
    xres = nc.dram_tensor("xres", [D, SS], FP8, kind="ExternalInput").ap()
    wqk = nc.dram_tensor("wqk", [D, 2 * FPC], FP8, kind="ExternalInput").ap()
    bqk = nc.dram_tensor("bqk", [128, 2], F32, kind="ExternalInput").ap()
    wv = nc.dram_tensor("wv", [D, FPC], FP8, kind="ExternalInput").ap()
    wvr = nc.dram_tensor("wvr", [D, FPC], FP8, kind="ExternalInput").ap()
    wout = nc.dram_tensor("wout", [D, D], BF16, kind="ExternalInput").ap()
    bob = nc.dram_tensor("bob", [128, D], F32, kind="ExternalInput").ap()
    ident = nc.dram_tensor("ident", [128, 128], BF16,
                           kind="ExternalInput").ap()
    onesr = nc.dram_tensor("onesr", [VW, HD], F32R,
                           kind="ExternalInput").ap()
    out = nc.dram_tensor("out", [SC, D], F32, kind="ExternalOutput").ap()

    with ExitStack() as ctx:
        tc = ctx.enter_context(tile.TileContext(nc))
        persist = ctx.enter_context(tc.tile_pool(name="persist", bufs=1))
        pexp = ctx.enter_context(tc.tile_pool(name="pexp", bufs=6))
        pwork = ctx.enter_context(tc.tile_pool(name="pwork", bufs=4))
        pbig = ctx.enter_context(tc.tile_pool(name="pbig", bufs=2, space="PSUM"))
        pso = ctx.enter_context(tc.tile_pool(name="pso", bufs=2, space="PSUM"))
        pps = ctx.enter_context(tc.tile_pool(name="pps", bufs=2, space="PSUM"))
        dram = ctx.enter_context(tc.tile_pool(name="dram", bufs=2, space="DRAM"))

        # ---------------- persistent SBUF ----------------
        # xt_sb[b]: [128, (i 8, s 2048)] fp8; xr_sb = fp8 residual
        xt_sb = [persist.tile([128, 8 * S], FP8, tag=f"xt{b}",
                              name=f"xt{b}") for b in range(B)]
        xt_r = [t.rearrange("p (i s) -> p i s", i=8) for t in xt_sb]
        xr_sb = [persist.tile([128, 8 * S], FP8, tag=f"xr{b}",
                              name=f"xr{b}") for b in range(B)]
        xr_r = [t.rearrange("p (i s) -> p i s", i=8) for t in xr_sb]
        wqk_sb = persist.tile([128, 8 * 2 * FPC], FP8, tag="wqk",
                              name="wqk_sb")
        wqk_r = wqk_sb.rearrange("p (i f) -> p i f", i=8)
        bqk_sb = persist.tile([128, 2], F32, tag="bqk", name="bqk")
        wv_sb = persist.tile([128, 8 * FPC], FP8, tag="wv", name="wv_sb")
        wv_r = wv_sb.rearrange("p (i f) -> p i f", i=8)
        wvr_sb = persist.tile([128, 8 * FPC], FP8, tag="wvr", name="wvr_sb")
        wvr_r = wvr_sb.rearrange("p (i f) -> p i f", i=8)
        wout_sb = persist.tile([128, 8 * D], BF16, tag="wout", name="wout_sb")
        wout_r = wout_sb.rearrange("p (i f) -> p i f", i=8)
        bob_sb = persist.tile([128, D], F32, tag="bob", name="bob")
        ident_sb = persist.tile([128, 128], BF16, tag="ident", name="ident")
        qk_sb = [[[persist.tile([128, 512], BF16, tag=f"qk{b}_{m}_{qn}",
                                name=f"qk{b}_{m}_{qn}") for qn in range(NQC)]
                  for m in range(2)]
                 for b in range(B)]
        # v2{hi,lo}[b][pi]: [128, 2*(2*VP)] fp8 - key-block pair pi; layout
        # [p, j(2), h(2), c(VP)], v at c 0:64; ones col 64: 1.0 in hi, 0 in lo
        v2hi = [[persist.tile([128, 2 * 2 * VP], FP8, tag=f"vh{b}_{pi}",
                              name=f"vh{b}_{pi}") for pi in range(NPAIR)]
                for b in range(B)]
        v2lo = [[persist.tile([128, 2 * 2 * VP], FP8, tag=f"vl{b}_{pi}",
                              name=f"vl{b}_{pi}") for pi in range(NPAIR)]
                for b in range(B)]
        v2hi_r = [[t.rearrange("p (j h c) -> p j h c", j=2, h=2)
                   for t in row] for row in v2hi]
        v2lo_r = [[t.rearrange("p (j h c) -> p j h c", j=2, h=2)
                   for t in row] for row in v2lo]
        att_sb = [[persist.tile([64, S], BF16, tag=f"att{b}_{h}",
                                name=f"att{b}_{h}") for h in range(HPC)]
                  for b in range(B)]
        ones32 = persist.tile([VW, HD], F32R, tag="ones32", name="ones32")
        nc.sync.dma_start(out=ones32, in_=onesr[:, :])
        ao_sb = [persist.tile([128, SC], BF16, tag=f"ao{i}", name=f"ao{i}")
                 for i in range(8)]
        part_sb = [persist.tile([128, 512], F32, tag=f"part{t}",
                                name=f"part{t}") for t in range(8)]
        for b in range(B):
            for pi in range(NPAIR):
                nc.gpsimd.memset(v2hi_r[b][pi][:, :, :, HD:VW], 1.0)
                nc.gpsimd.memset(v2lo_r[b][pi][:, :, :, HD:VW], 0.0)

        # ---------------- loads (ordered by first use) ----------------
        def load_x(dst_r, src, b, s0, s1, eng=None):
            # (p, i, s) -> src[i*128 + p, b*S + s]
            ap = bass.AP(tensor=src.tensor, offset=b * S + s0,
                         ap=[[SS, 128], [128 * SS, 8], [1, s1 - s0]])
            (eng or nc.sync).dma_start(out=dst_r[b][:, :, s0:s1], in_=ap)

        wqk_src = bass.AP(tensor=wqk.tensor, offset=0,
                          ap=[[2 * FPC, 128], [128 * 2 * FPC, 8], [1, 2 * FPC]])
        nc.sync.dma_start(out=wqk_r, in_=wqk_src)
        load_x(xt_r, xt, 0, 0, 1024)
        load_x(xr_r, xres, 0, 0, 1024)
        nc.sync.dma_start(out=bqk_sb, in_=bqk[:, :])
        load_x(xt_r, xt, 0, 1024, 2048)
        load_x(xr_r, xres, 0, 1024, 2048)
        wv_src = bass.AP(tensor=wv.tensor, offset=0,
                         ap=[[FPC, 128], [128 * FPC, 8], [1, FPC]])
        nc.sync.dma_start(out=wv_r, in_=wv_src)
        wvr_src = bass.AP(tensor=wvr.tensor, offset=0,
                          ap=[[FPC, 128], [128 * FPC, 8], [1, FPC]])
        nc.sync.dma_start(out=wvr_r, in_=wvr_src)
        nc.sync.dma_start(out=ident_sb, in_=ident[:, :])
        load_x(xt_r, xt, 1, 0, 2048)
        load_x(xr_r, xres, 1, 0, 2048)
        wout_src = bass.AP(tensor=wout.tensor, offset=0,
                           ap=[[D, 128], [128 * D, 8], [1, D]])
        nc.sync.dma_start(out=wout_r, in_=wout_src)
        nc.sync.dma_start(out=bob_sb, in_=bob[:, :])

        a2a_in = [dram.tile([8, HD, SC], BF16, tag=f"a2a_in{h}",
                            name=f"a2a_in{h}", bufs=1) for h in range(HPC)]
        a2a_out = [dram.tile([8, HD, SC], BF16, tag=f"a2a_out{h}",
                             name=f"a2a_out{h}", bufs=1) for h in range(HPC)]

        def emit_a2a(h):
            if with_collective:
                nc.gpsimd.collective_compute(
                    "AllToAll", mybir.AluOpType.bypass,
                    replica_groups=[list(range(8))],
                    ins=[a2a_in[h][:, :, :].opt()],
                    outs=[a2a_out[h][:, :, :].opt()])

        # ------------- projections + attention, interleaved -------------
        def emit_qk(b, m, qn):
            ps = pps.tile([128, 512], F32, tag="ps", name="ps_qk")
            for i in range(4):
                nc.tensor.matmul(
                    ps,
                    wqk_r[:, 2 * i:2 * i + 2, m * 128:(m + 1) * 128],
                    xt_r[b][:, 2 * i:2 * i + 2, qn * 512:(qn + 1) * 512],
                    start=(i == 0), stop=False, perf_mode=DR)
            for i in range(4):
                nc.tensor.matmul(
                    ps,
                    wqk_r[:, 2 * i:2 * i + 2, m * 128:(m + 1) * 128],
                    xr_r[b][:, 2 * i:2 * i + 2, qn * 512:(qn + 1) * 512],
                    start=False, stop=(i == 3), perf_mode=DR)
            nc.vector.tensor_scalar(
                qk_sb[b][m][qn], ps,
                (0.125 / WSC) if m == 0 else (1.0 / WSC),
                bqk_sb[:, m:m + 1], op0=MUL, op1=ADD)

        # v projection, operand-swapped: out [128 feat, 512 seq] in one
        # 12-matmul pass per (b, cq); transposed back per 128-seq block
        vfm_store = {}

        def emit_v_mm(b, cq):
            ps = pps.tile([128, 512], F32, tag="ps", name="ps_v")
            terms = [(wv_r, xt_r), (wvr_r, xt_r), (wv_r, xr_r)]
            n = 0
            for wsrc, xsrc in terms:
                for i in range(4):
                    nc.tensor.matmul(
                        ps,
                        wsrc[:, 2 * i:2 * i + 2, :],
                        xsrc[b][:, 2 * i:2 * i + 2,
                                cq * 512:(cq + 1) * 512],
                        start=(n == 0), stop=(n == 11), perf_mode=DR)
                    n += 1
            vfm = pwork.tile([128, 512], BF16, tag="vfm", name="vfm")
            if (b * 4 + cq) % 2:
                nc.scalar.copy(vfm, ps)
            else:
                nc.vector.tensor_copy(vfm, ps)
            vfm_store[(b, cq)] = vfm

        def emit_v_tr(b, cq):
            vfm = vfm_store.pop((b, cq))
            for s4 in range(4):
                sn = cq * 4 + s4
                pi, j = sn // 2, sn % 2
                pst = pps.tile([128, 128], BF16, tag="ps", name="pst")
                nc.tensor.transpose(
                    pst, vfm[:, s4 * 128:(s4 + 1) * 128], ident_sb)
                pst_r = pst.rearrange("p (h c) -> p h c", h=2)
                hi = v2hi_r[b][pi][:, j, :, 0:HD]
                nc.vector.tensor_copy(hi, pst_r)
                nc.vector.tensor_tensor(
                    v2lo_r[b][pi][:, j, :, 0:HD], pst_r, hi, op=SUB)

        def emit_attn(b, h, qh, fillers=()):
            # fillers: [(slot, thunk)]; popped at kb >= slot, so a filler's
            # products may only be consumed at kb >= slot (or later groups)
            fillers = sorted([e if isinstance(e, tuple) else (0, e)
                              for e in fillers], key=lambda e: e[0])
            pb = h * 64
            ps_o = [pso.tile([VW, 512], F32, tag="pso",
                             name=f"ps_o{q2}") for q2 in range(2)]

            def emit_av(pi, exv):
                for q2 in range(2):
                    nc.tensor.matmul(
                        ps_o[q2],
                        v2hi_r[b][pi][:, :, h, 0:VW],
                        exv[:, :, q2 * 512:(q2 + 1) * 512],
                        start=(pi == 0), stop=False, perf_mode=DR)
                    nc.tensor.matmul(
                        ps_o[q2],
                        v2lo_r[b][pi][:, :, h, 0:VW],
                        exv[:, :, q2 * 512:(q2 + 1) * 512],
                        start=False, stop=(pi == NPAIR - 1), perf_mode=DR)

            # attn@V for pair pi is emitted one pair late so PE never
            # hard-waits on the pair's last exp
            pend = None
            for pi in range(NPAIR):
                ex = pexp.tile([128, 2 * 1024], FP8, tag="expT",
                               name="expT")
                exv = ex.rearrange("p (j q) -> p j q", j=2)
                exu = ex.bitcast(U8).rearrange("p (j q) -> p j q", j=2)
                for j in range(2):
                    kb = 2 * pi + j
                    while fillers and fillers[0][0] <= kb:
                        fillers.pop(0)[1]()
                    ps_s = pbig.tile([128, 1024], F32, tag="scores",
                                     name="ps_s")
                    for q2 in range(2):
                        qc = qh * 2 + q2
                        nc.tensor.matmul(
                            ps_s[:, q2 * 512:(q2 + 1) * 512],
                            qk_sb[b][1][kb // 4][pb:pb + 64,
                                                 (kb % 4) * 128:
                                                 (kb % 4 + 1) * 128],
                            qk_sb[b][0][qc][pb:pb + 64, :],
                            start=True, stop=True)
                    if EXP_SCHED[kb] == "a":
                        nc.scalar.activation(exv[:, j, :], ps_s, Exp)
                    else:
                        nc.vector.tensor_scalar(
                            exu[:, j, :], ps_s, SCH_A, SCH_B,
                            op0=MUL, op1=ADD)
                    if j == 1 and pend is not None:
                        emit_av(*pend)
                pend = (pi, exv)
            emit_av(*pend)
            for _, f in fillers:
                f()
            # normalization head: recip + otmp now; bc+mult+ship deferred
            rec_s = pwork.tile([VW, 1024], F32R, tag="rec", name="rec_s")
            with nc.allow_low_precision(
                    reason="softmax denom recip rounded to f32r "
                           "for the PE broadcast"):
                for q2 in range(2):
                    nc.vector.reciprocal(
                        rec_s[HD:VW, q2 * 512:(q2 + 1) * 512],
                        ps_o[q2][HD:VW, :])
            otmps = []
            for q2 in range(2):
                otmp = pwork.tile([HD, 512], F32, tag="otmp",
                                  name="otmp")
                nc.scalar.copy(otmp, ps_o[q2][0:HD, :])
                otmps.append(otmp)

            def finish(q2):
                qc = qh * 2 + q2
                bc_ps = pps.tile([HD, 512], F32, tag="ps", name="bc_ps")
                nc.tensor.matmul(
                    bc_ps,
                    ones32[HD:VW, :],
                    rec_s[HD:VW, q2 * 512:(q2 + 1) * 512],
                    start=True, stop=True)
                nc.vector.tensor_tensor(
                    att_sb[b][h][:, qc * 512:(qc + 1) * 512],
                    otmps[q2], bc_ps[:, :], op=MUL)
                if qh == 1 and q2 == 1:
                    # (b,h) complete: ship the whole row in one DMA
                    nc.sync.dma_start(
                        out=a2a_in[h][b * 4:(b + 1) * 4, :, :].rearrange(
                            "j p s -> p j s"),
                        in_=att_sb[b][h].rearrange("p (j s) -> p j s", j=4))

            return [lambda: finish(0), lambda: finish(1)]

        srcb = a2a_out if with_collective else a2a_in

        def emit_unload(hi):
            for jj in range(4):
                nc.sync.dma_start(
                    out=ao_sb[4 * hi + jj],
                    in_=srcb[hi][2 * jj:2 * jj + 2, :, :].rearrange(
                        "j p s -> (j p) s"))

        # output projection tile t = sm*2 + en (bf16)
        def emit_out1(t):
            sm, en = t // 2, t % 2
            ps = pps.tile([128, 512], F32, tag="ps", name="ps_out1")
            for kk in range(4):
                nc.tensor.matmul(
                    ps, ao_sb[kk][:, sm * 128:(sm + 1) * 128],
                    wout_sb[:, kk * D + en * 512:kk * D + (en + 1) * 512],
                    start=(kk == 0), stop=(kk == 3))
            nc.vector.scalar_tensor_tensor(
                part_sb[t], ps, 1.0 / WSC,
                bob_sb[:, en * 512:(en + 1) * 512], op0=MUL, op1=ADD)

        def emit_out2(t):
            sm, en = t // 2, t % 2
            ps = pps.tile([128, 512], F32, tag="ps", name="ps_out2")
            for kk in range(4, 8):
                nc.tensor.matmul(
                    ps, ao_sb[kk][:, sm * 128:(sm + 1) * 128],
                    wout_sb[:, kk * D + en * 512:kk * D + (en + 1) * 512],
                    start=(kk == 4), stop=(kk == 7))
            osb = pwork.tile([128, 512], F32, tag="outsb", name="osb")
            nc.vector.scalar_tensor_tensor(
                osb, ps, 1.0 / WSC, part_sb[t], op0=MUL, op1=ADD)
            nc.sync.dma_start(
                out=out[sm * 128:(sm + 1) * 128, en * 512:(en + 1) * 512],
                in_=osb)

        def F(fn, *a):
            return lambda: fn(*a)

        # Filler safety rule: a filler popped at kb-slot i is emitted just
        # before slot i's scores, so anything it produces may only be
        # consumed at kb >= i (or by a later group).
        emit_qk(0, 1, 0)
        emit_qk(0, 1, 1)
        emit_qk(0, 0, 0)
        emit_qk(0, 0, 1)
        fin = emit_attn(0, 0, 0, fillers=(
            [(4 * cq, F(emit_v_mm, 0, cq)) for cq in range(4)]
            + [(4 * cq + 1, F(emit_v_tr, 0, cq)) for cq in range(4)]
            + [(6, F(emit_qk, 0, 1, 2)), (10, F(emit_qk, 0, 1, 3))]))
        fin = emit_attn(0, 1, 0, fillers=[
            (1, fin[0]), (3, fin[1]),
            (5, F(emit_qk, 0, 0, 2)), (9, F(emit_qk, 0, 0, 3))])
        fin = emit_attn(0, 0, 1, fillers=(
            [(1, fin[0]), (3, fin[1])]
            + [(i * 2, F(emit_qk, 1, 1, qn))
               for i, qn in enumerate(range(NQC))]
            + [(8, F(emit_qk, 1, 0, 0)), (10, F(emit_qk, 1, 0, 1))]))
        fin = emit_attn(0, 1, 1, fillers=(
            [(1, fin[0]), (3, fin[1])]
            + [(0, F(emit_v_mm, 1, 0)), (2, F(emit_v_tr, 1, 0)),
               (4, F(emit_v_mm, 1, 1)), (6, F(emit_v_tr, 1, 1))]
            + [(14, F(emit_qk, 1, 0, 2)), (15, F(emit_qk, 1, 0, 3))]))
        fin = emit_attn(1, 0, 0, fillers=(
            [(1, fin[0]), (3, fin[1])]
            + [(0, F(emit_v_mm, 1, 2)), (2, F(emit_v_tr, 1, 2)),
               (4, F(emit_v_mm, 1, 3)), (6, F(emit_v_tr, 1, 3))]))
        fin = emit_attn(1, 0, 1, fillers=[(1, fin[0]), (3, fin[1])])
        # (1,1,0) flushes the last even-head finishers at slots 1/3, so
        # a2a #0 + unload fire mid-group and out1 spreads over two groups
        fin = emit_attn(1, 1, 0, fillers=(
            [(1, fin[0]), (3, fin[1]),
             (4, F(emit_a2a, 0)), (5, F(emit_unload, 0))]
            + [(6 + 2 * t, F(emit_out1, t)) for t in range(4)]))
        fin2 = emit_attn(1, 1, 1, fillers=(
            [(1, fin[0]), (3, fin[1])]
            + [(2 * t, F(emit_out1, 4 + t)) for t in range(4)]))
        for f in fin2:
            f()
        emit_a2a(1)
        emit_unload(1)
        for t in range(8):
            emit_out2(t)

    nc.compile()
    return nc


_NC_CACHE = {}


def _get_nc(with_collective: bool = True):
    key = bool(with_collective)
    if key not in _NC_CACHE:
        _NC_CACHE[key] = _build_nc(with_collective)
    return _NC_CACHE[key]


def make_in_maps(x, w_qkv, b_qkv, w_out, b_out):
    """Host-side sharding/prep. Returns per-core input dicts."""
    x = np.asarray(x, dtype=np.float32)
    w_qkv = np.asarray(w_qkv, dtype=np.float32)
    b_qkv = np.asarray(b_qkv, dtype=np.float32)
    w_out = np.asarray(w_out, dtype=np.float32)
    b_out = np.asarray(b_out, dtype=np.float32)

    wq = w_qkv[0:D].reshape(H, HD, D)
    wk = w_qkv[D:2 * D].reshape(H, HD, D)
    wv = w_qkv[2 * D:3 * D].reshape(H, HD, D)
    bq = b_qkv[0:D].reshape(H, HD)
    bk = b_qkv[D:2 * D].reshape(H, HD)
    scale = 1.0 / np.sqrt(HD)

    perm = np.concatenate(
        [np.arange(h * HD, (h + 1) * HD) for h in range(0, H, 2)]
        + [np.arange(h * HD, (h + 1) * HD) for h in range(1, H, 2)])
    wout_t = np.ascontiguousarray(w_out.T[perm]).astype(NPBF16)
    # v bias passes through attention unchanged (softmax rows sum to 1),
    # so its out-proj image folds into the output bias
    bv_full = b_qkv[2 * D:3 * D]
    bob_vec = b_out + w_out @ bv_full
    bob = np.ascontiguousarray(
        np.broadcast_to(bob_vec, (128, D))).astype(np.float32)

    # [d, 4096] stacked batch-major; fp8 hi + fp8 residual
    xt_f32 = np.ascontiguousarray(
        np.concatenate([x[0].T, x[1].T], axis=1))
    xt_all = xt_f32.astype(NPFP8)
    xres_all = (xt_f32 - xt_all.astype(np.float32)).astype(NPFP8)

    in_maps = []
    for c in range(NCORES):
        hs = slice(c * HPC, (c + 1) * HPC)
        wq_c = (wq[hs].reshape(FPC, D) * WSC).T
        wk_c = (wk[hs].reshape(FPC, D) * WSC).T
        wqk_c = np.concatenate([wq_c, wk_c], axis=1).astype(NPFP8)
        bqk_c = np.concatenate([bq[hs].reshape(FPC) * scale,
                                bk[hs].reshape(FPC)])
        bqk_c = np.ascontiguousarray(
            bqk_c.reshape(2, 128).T).astype(np.float32)
        wv_f = (wv[hs].reshape(FPC, D) * WSC).T
        wv_c = wv_f.astype(NPFP8)
        wvr_c = (wv_f - wv_c.astype(np.float32)).astype(NPFP8)
        in_maps.append({
            "onesr": np.ones((VW, HD), dtype=np.float32),
            "ident": np.eye(128, dtype=np.float32).astype(NPBF16),
            "xt": xt_all,
            "xres": xres_all,
            "wqk": np.ascontiguousarray(wqk_c),
            "bqk": bqk_c,
            "wv": np.ascontiguousarray(wv_c),
            "wvr": np.ascontiguousarray(wvr_c),
            "wout": wout_t,
            "bob": bob,
        })
    return in_maps


def assemble_output(results):
    out = np.empty((B, S, D), dtype=np.float32)
    for c in range(NCORES):
        b, sg = c // 4, c % 4
        out[b, sg * SC:(sg + 1) * SC, :] = results[c]["out"]
    return out


def kernel(x, mask, w_qkv, b_qkv, w_out, b_out):
    nc = _get_nc(True)
    in_maps = make_in_maps(x, w_qkv, b_qkv, w_out, b_out)
    res = run_bass_kernel_spmd(nc, in_maps, core_ids=list(range(NCORES)))
    return assemble_output(res.results)
